# revision 37
# baseline (speedup 1.0000x reference)
"""Trainium2 Bass kernel for AIRS-GNN (4-layer GAT + readout) on 8 NeuronCores.

Self-contained: hardcodes all shapes/sharding. Host side does only integer
index manipulation (edge partitioning, padding, one-hot layout) plus constant
table construction; all floating-point math runs on device.

Sharding: nodes are sharded contiguously across the 8 cores (6250/core,
padded to 6272 = 49*128 rows).  Edges are owned by the core of their dst
node (pull-based aggregation).  Per layer the bf16 node-feature table
(h @ W) is AllGathered, then each core gathers the rows for its edges with
dma_gather and scatter-adds messages into per-window PSUM via one-hot
matmuls.  Softmax normalization is applied after aggregation (denominators
ride along as 4 extra matmul columns); the segment-max shift of the
reference cancels algebraically and is skipped.
"""

import os
import numpy as np
import ml_dtypes

# ---------------- problem constants (from spec) ----------------
N, E, F, HID, HEADS, L, B, R = 50000, 400000, 64, 256, 4, 4, 16, 8
C = HID // HEADS  # 64
NCORES = 8
SHARD = N // NCORES            # 6250
NTILES = (SHARD + 127) // 128  # 49
PADSHARD = NTILES * 128        # 6272
HALF = PADSHARD * (NCORES // 2)  # 25088 rows per half-table
LN_EPS = 1e-5
NEG_SLOPE = 0.2

BF16 = ml_dtypes.bfloat16

SIM_MODE = False  # debug: replace sim-unsupported Gelu with Identity
STAGE = 99  # debug: truncate graph after this stage (99 = full)

_cache = {}


def _posenc(n, d):
    pos = np.arange(n, dtype=np.float32)[:, None]
    i = np.arange(d, dtype=np.float32)[None, :]
    rates = (1.0 / 10000.0 ** (2.0 * np.floor(i / 2.0) / d)).astype(np.float32)
    ang = pos * rates
    return np.where(np.arange(d)[None, :] % 2 == 0, np.sin(ang), np.cos(ang)).astype(
        np.float32
    )


def _wrap16(a):
    """dma_gather index layout: idxs[p, s] = a[s*16 + p], replicated to 128 parts."""
    assert a.size % 16 == 0
    w = a.reshape(-1, 16).T.astype(np.int16)  # [16, S/16]
    return np.tile(w, (8, 1))  # [128, S/16]


def _prep(inputs):
    """Host-side integer prep. Returns (in_maps, struct)."""
    ei = np.asarray(inputs["edge_index"])
    src = np.concatenate([np.asarray(ei[0]), np.arange(N)]).astype(np.int64)
    dst = np.concatenate([np.asarray(ei[1]), np.arange(N)]).astype(np.int64)

    core = dst // SHARD
    dloc = dst - core * SHARD
    win = dloc // 128
    drel = dloc - win * 128
    grow = (src // SHARD) * PADSHARD + (src % SHARD)  # padded global table row
    half = (grow >= HALF).astype(np.int64)

    # tiles needed per (window, half): max over cores
    cnt = np.zeros((NCORES, NTILES, 2), np.int64)
    np.add.at(cnt, (core, win, half), 1)
    T = np.maximum((cnt.max(axis=0) + 127) // 128, 1)  # [NTILES, 2]
    T_lo, T_hi = T[:, 0], T[:, 1]
    Tw = T_lo + T_hi
    wslot0 = np.concatenate([[0], np.cumsum(Tw * 128)])
    TOTSLOT = int(wslot0[-1])

    gidx_maps, sdidx_maps, ids_maps = [], [], []
    for c in range(NCORES):
        sel = core == c
        wc, dc, gc, hc = win[sel], drel[sel], grow[sel], half[sel]
        key = wc * 2 + hc
        order = np.argsort(key, kind="stable")
        wc, dc, gc, hc = wc[order], dc[order], gc[order], hc[order]
        ks = key[order]
        grp_start = np.searchsorted(ks, np.arange(NTILES * 2), side="left")
        pos = np.arange(ks.size) - grp_start[ks]
        slot = wslot0[wc] + np.where(hc == 1, T_lo[wc] * 128, 0) + pos
        assert slot.max() < TOTSLOT

        g_val = np.zeros(TOTSLOT, np.int64)          # pad -> row 0 (valid)
        g_val[slot] = gc - hc * HALF
        sd_val = np.zeros(TOTSLOT, np.int64)
        sd_val[slot] = wc * 128 + dc
        id_val = np.full(TOTSLOT, -1.0, np.float32)  # pad -> -1 (no one-hot match)
        id_val[slot] = dc

        gcols, sdcols, idcols = [], [], []
        for w in range(NTILES):
            s0 = wslot0[w]
            nlo, nhi = int(T_lo[w]) * 128, int(T_hi[w]) * 128
            gcols.append(_wrap16(g_val[s0 : s0 + nlo]))
            gcols.append(_wrap16(g_val[s0 + nlo : s0 + nlo + nhi]))
            sdcols.append(_wrap16(sd_val[s0 : s0 + nlo + nhi]))
            idcols.append(id_val[s0 : s0 + nlo + nhi].reshape(-1, 128).T.astype(BF16))
        gidx_maps.append(np.concatenate(gcols, axis=1))
        sdidx_maps.append(np.concatenate(sdcols, axis=1))
        ids_maps.append(np.concatenate(idcols, axis=1))

    # ---------------- dense/static per-core tensors ----------------
    x = np.asarray(inputs["x"], np.float32)
    region_ids = np.asarray(inputs["region_ids"]).astype(np.int64)
    batch = np.asarray(inputs["batch"]).astype(np.int64)
    pe = _posenc(N, F)

    counts = np.bincount(batch, minlength=B).astype(np.float32)
    inv_cnt = (1.0 / np.maximum(counts, 1.0)).astype(np.float32)[:, None]

    w_in = np.asarray(inputs["in_proj_w"], np.float32)  # [3F, HID]
    gat_w = np.asarray(inputs["gat_w"], np.float32)     # [L, HID, HID]
    # [128, L*2, HID]: chunk (l,k) at [:, l*2+k, :]
    gatw_h = np.ascontiguousarray(
        gat_w.reshape(L, 2, 128, HID).transpose(2, 0, 1, 3).reshape(128, L * 2, HID)
    ).astype(BF16)

    def rep128(a, d):  # [L, HID] -> [128, L, HID] replicated, dtype d
        return np.ascontiguousarray(
            np.broadcast_to(np.asarray(a, np.float32)[None, :, :], (128, L, HID))
        ).astype(d)

    asr_h = rep128(np.asarray(inputs["att_src"], np.float32).reshape(L, HID), BF16)
    adr_h = rep128(np.asarray(inputs["att_dst"], np.float32).reshape(L, HID), BF16)
    gbr_h = rep128(inputs["gat_b"], np.float32)
    lgr_h = rep128(inputs["ln_g"], np.float32)
    lbr_h = rep128(inputs["ln_b"], BF16)
    ipb_h = np.ascontiguousarray(
        np.asarray(inputs["in_proj_b"], np.float32).reshape(2, 128).T
    )  # [128, 2]
    row1_h = np.ascontiguousarray(
        np.asarray(inputs["ro_w1"], np.float32).reshape(2, 128, HID).transpose(1, 0, 2)
    )  # [128, 2, HID]
    row2_h = np.ascontiguousarray(
        np.asarray(inputs["ro_w2"], np.float32).reshape(2, 128, HID).transpose(1, 0, 2)
    )

    in_maps = []
    for c in range(NCORES):
        lo, hi = c * SHARD, (c + 1) * SHARD
        xT = np.zeros((F, PADSHARD), np.float32)
        xT[:, :SHARD] = x[lo:hi].T
        peT = np.zeros((F, PADSHARD), np.float32)
        peT[:, :SHARD] = pe[lo:hi].T
        rT = np.zeros((R, PADSHARD), np.float32)
        rT[region_ids[lo:hi], np.arange(SHARD)] = 1.0
        ohb = np.zeros((128, NTILES, B), BF16)
        p_all = np.arange(SHARD)
        ohb[p_all % 128, p_all // 128, batch[lo:hi]] = 1.0

        m = {
            "xT": xT,
            "peT": peT,
            "rT": rT,
            "gidx": gidx_maps[c].astype(np.int16),
            "sdidx": sdidx_maps[c].astype(np.int16),
            "ids": ids_maps[c].astype(BF16),
            "ohb": ohb,
            "w_x": w_in[:F].copy(),
            "w_r2": w_in[F : 2 * F].copy(),
            "w_p": w_in[2 * F :].copy(),
            "embT": np.asarray(inputs["region_emb_w"], np.float32).T.copy(),
            "ipb": ipb_h,
            "gatw": gatw_h,
            "asr": asr_h,
            "adr": adr_h,
            "gbr": gbr_h,
            "lgr": lgr_h,
            "lbr": lbr_h,
            "row1": row1_h,
            "row2": row2_h,
            "b1r": np.ascontiguousarray(
                np.broadcast_to(np.asarray(inputs["ro_b1"], np.float32)[None, :], (B, HID))
            ),
            "b2r": np.ascontiguousarray(
                np.broadcast_to(np.asarray(inputs["ro_b2"], np.float32)[None, :], (B, HID))
            ),
            "invc": inv_cnt,
            "iota": np.ascontiguousarray(
                np.broadcast_to(np.arange(128, dtype=np.float32)[None, :], (128, 128))
            ).astype(BF16),
            "identb": np.eye(128, dtype=np.float32).astype(BF16),
            "identf": np.eye(128, dtype=np.float32),
        }
        in_maps.append(m)

    struct = {
        "T_lo": [int(t) for t in T_lo],
        "T_hi": [int(t) for t in T_hi],
        "GCOLS": int(gidx_maps[0].shape[1]),
        "SDCOLS": int(sdidx_maps[0].shape[1]),
        "IDCOLS": int(ids_maps[0].shape[1]),
    }
    return in_maps, struct


def _build(struct):
    """Build the Bass graph (identical for all cores)."""
    import concourse.bass as bass  # noqa: F401
    import concourse.tile as tile
    from concourse import bacc, mybir

    dt = mybir.dt
    AX = mybir.AxisListType
    OP = mybir.AluOpType
    ACT = mybir.ActivationFunctionType

    T_lo, T_hi = struct["T_lo"], struct["T_hi"]
    Tw = [a + b for a, b in zip(T_lo, T_hi)]
    TMAX = max(Tw)

    nc = bacc.Bacc(
        "TRN2", target_bir_lowering=False, debug=False, num_devices=NCORES
    )
    RG = [list(range(NCORES))]

    def din(name, shape, d=dt.float32):
        return nc.dram_tensor(name, shape, d, kind="ExternalInput")

    t_xT = din("xT", [F, PADSHARD])
    t_peT = din("peT", [F, PADSHARD])
    t_rT = din("rT", [R, PADSHARD])
    t_gidx = din("gidx", [128, struct["GCOLS"]], dt.int16)
    t_sdidx = din("sdidx", [128, struct["SDCOLS"]], dt.int16)
    t_ids = din("ids", [128, struct["IDCOLS"]], dt.bfloat16)
    t_ohb = din("ohb", [128, NTILES, B], dt.bfloat16)
    t_wx = din("w_x", [F, HID])
    t_wr2 = din("w_r2", [F, HID])
    t_wp = din("w_p", [F, HID])
    t_embT = din("embT", [F, R])
    t_ipb = din("ipb", [128, 2])
    t_gatw = din("gatw", [128, L * 2, HID], dt.bfloat16)
    t_asr = din("asr", [128, L, HID], dt.bfloat16)
    t_adr = din("adr", [128, L, HID], dt.bfloat16)
    t_gbr = din("gbr", [128, L, HID])
    t_lgr = din("lgr", [128, L, HID])
    t_lbr = din("lbr", [128, L, HID], dt.bfloat16)
    t_row1 = din("row1", [128, 2, HID])
    t_row2 = din("row2", [128, 2, HID])
    t_b1r = din("b1r", [B, HID])
    t_b2r = din("b2r", [B, HID])
    t_invc = din("invc", [B, 1])
    t_iota = din("iota", [128, 128], dt.bfloat16)
    t_identb = din("identb", [128, 128], dt.bfloat16)
    t_identf = din("identf", [128, 128])

    t_out = nc.dram_tensor("out", [B, HID], dt.float32, kind="ExternalOutput")

    # static per-window offsets (in idx columns / id columns)
    gcall = [0]
    for w in range(NTILES):
        gcall.append(gcall[-1] + T_lo[w] * 8)
        gcall.append(gcall[-1] + T_hi[w] * 8)
    sdoff = [0]
    idoff = [0]
    for w in range(NTILES):
        sdoff.append(sdoff[-1] + Tw[w] * 8)
        idoff.append(idoff[-1] + Tw[w])

    with tile.TileContext(nc) as tc:
        with (
            tc.tile_pool(name="const", bufs=1) as cpool,
            tc.tile_pool(name="dram", bufs=1, space="DRAM") as dpool,
            tc.tile_pool(name="persist", bufs=1) as ppool,
        ):
            def load(t, shape, d=dt.float32):
                tl = cpool.tile(shape, d, name=t.name + "_sb")
                nc.sync.dma_start(tl[:], t.ap())
                return tl

            gidx_sb = load(t_gidx, [128, struct["GCOLS"]], dt.int16)
            sdidx_sb = load(t_sdidx, [128, struct["SDCOLS"]], dt.int16)
            ids_sb = load(t_ids, [128, struct["IDCOLS"]], dt.bfloat16)
            ohb_sb = load(t_ohb, [128, NTILES, B], dt.bfloat16)
            ipb_sb = load(t_ipb, [128, 2])
            gatw_sb = load(t_gatw, [128, L * 2, HID], dt.bfloat16)
            asr_sb = load(t_asr, [128, L, HID], dt.bfloat16)
            adr_sb = load(t_adr, [128, L, HID], dt.bfloat16)
            gbr_sb = load(t_gbr, [128, L, HID])
            lgr_sb = load(t_lgr, [128, L, HID])
            lbr_sb = load(t_lbr, [128, L, HID], dt.bfloat16)
            row1_sb = load(t_row1, [128, 2, HID])
            row2_sb = load(t_row2, [128, 2, HID])
            b1r_sb = load(t_b1r, [B, HID])
            b2r_sb = load(t_b2r, [B, HID])
            invc_sb = load(t_invc, [B, 1])
            iota_sb = load(t_iota, [128, 128], dt.bfloat16)
            identb_sb = load(t_identb, [128, 128], dt.bfloat16)
            identf_sb = load(t_identf, [128, 128])

            zero1 = cpool.tile([128, 1], dt.float32, name="zero1")
            nc.vector.memset(zero1[:], 0.0)
            eps1 = cpool.tile([128, 1], dt.float32, name="eps1")
            nc.vector.memset(eps1[:], LN_EPS)

            hT = ppool.tile([128, 2 * PADSHARD], dt.bfloat16, name="hT")
            h_sb = ppool.tile([128, NTILES, HID], dt.bfloat16, name="h_sb")
            var_sb = ppool.tile([128, NTILES], dt.float32, name="var_sb")
            rstd_sb = ppool.tile([128, NTILES], dt.float32, name="rstd_sb")

            in_cc = [
                dpool.tile([PADSHARD, HID], dt.bfloat16, name=f"incc{l}")
                for l in range(L)
            ]
            out_cc = [
                dpool.tile(
                    [NCORES * PADSHARD, HID], dt.bfloat16, name=f"outcc{l}",
                    addr_space="Shared",
                )
                for l in range(L)
            ]
            s_dram = [
                dpool.tile([PADSHARD, 128], dt.bfloat16, name=f"sdram{l}")
                for l in range(L)
            ]
            ar_in = dpool.tile([B, HID], dt.float32, name="ar_in")
            ar_out = dpool.tile([B, HID], dt.float32, name="ar_out", addr_space="Shared")

            # ---- stage A: input projection -> hT (bf16) ----
            with (
                tc.tile_pool(name="aproj", bufs=1) as apool,
                tc.tile_pool(name="apsum", bufs=2, space="PSUM") as appool,
            ):
                xT_sb = apool.tile([F, PADSHARD], dt.float32, name="xT_sb")
                nc.sync.dma_start(xT_sb[:], t_xT.ap())
                peT_sb = apool.tile([F, PADSHARD], dt.float32, name="peT_sb")
                nc.sync.dma_start(peT_sb[:], t_peT.ap())
                rT_sb = apool.tile([R, PADSHARD], dt.float32, name="rT_sb")
                nc.sync.dma_start(rT_sb[:], t_rT.ap())
                wx_sb = apool.tile([F, HID], dt.float32, name="wx_sb")
                nc.sync.dma_start(wx_sb[:], t_wx.ap())
                wr2_sb = apool.tile([F, HID], dt.float32, name="wr2_sb")
                nc.sync.dma_start(wr2_sb[:], t_wr2.ap())
                wp_sb = apool.tile([F, HID], dt.float32, name="wp_sb")
                nc.sync.dma_start(wp_sb[:], t_wp.ap())
                embT_sb = apool.tile([F, R], dt.float32, name="embT_sb")
                nc.sync.dma_start(embT_sb[:], t_embT.ap())

                ew_ps = appool.tile([R, HID], dt.float32, name="ew_ps")
                nc.tensor.matmul(ew_ps[:], embT_sb[:], wr2_sb[:])
                ew_sb = apool.tile([R, HID], dt.float32, name="ew_sb")
                nc.vector.tensor_copy(ew_sb[:], ew_ps[:])

                NBLK = 512
                nblocks = (PADSHARD + NBLK - 1) // NBLK
                for k in range(2):
                    fs = slice(k * 128, (k + 1) * 128)
                    for nb in range(nblocks):
                        c0 = nb * NBLK
                        cw = min(NBLK, PADSHARD - c0)
                        ps = appool.tile([128, NBLK], dt.float32, name="aps", tag="aps")
                        nc.tensor.matmul(
                            ps[:, :cw], wx_sb[:, fs], xT_sb[:, c0 : c0 + cw],
                            start=True, stop=False,
                        )
                        nc.tensor.matmul(
                            ps[:, :cw], ew_sb[:, fs], rT_sb[:, c0 : c0 + cw],
                            start=False, stop=False,
                        )
                        nc.tensor.matmul(
                            ps[:, :cw], wp_sb[:, fs], peT_sb[:, c0 : c0 + cw],
                            start=False, stop=True,
                        )
                        nc.vector.tensor_scalar_add(
                            hT[:, k * PADSHARD + c0 : k * PADSHARD + c0 + cw],
                            ps[:, :cw],
                            ipb_sb[:, k : k + 1],
                        )

            # ---- GAT layers ----
            with (
                tc.tile_pool(name="b1", bufs=3) as b1pool,
                tc.tile_pool(name="b1ps", bufs=2, space="PSUM") as b1ps,
                tc.tile_pool(name="win", bufs=3) as wpool,
                tc.tile_pool(name="win2", bufs=2) as wpool2,
                tc.tile_pool(name="wps", bufs=2, space="PSUM") as wps,
                tc.tile_pool(name="tps", bufs=2, space="PSUM") as tps,
            ):
                _sub = 20 <= STAGE < 40
                nlayers = 0 if STAGE < 1 else (1 if (STAGE < 6 or _sub) else L)
                for l in range(nlayers):
                    # --- B1: h2 = h @ W_l; s_dst; feature table ---
                    for w in range(NTILES):
                        ps2 = b1ps.tile([128, HID], dt.float32, name="h2ps", tag="h2ps")
                        for k in range(2):
                            nc.tensor.matmul(
                                ps2[:],
                                hT[:, k * PADSHARD + w * 128 : k * PADSHARD + (w + 1) * 128],
                                gatw_sb[:, l * 2 + k, :],
                                start=(k == 0),
                                stop=(k == 1),
                            )
                        g_t = b1pool.tile([128, HID], dt.bfloat16, name="g_t", tag="g_t")
                        nc.vector.tensor_copy(g_t[:], ps2[:])
                        nc.sync.dma_start(in_cc[l][w * 128 : (w + 1) * 128, :], g_t[:])
                        sdt_t = b1pool.tile([128, HID], dt.bfloat16, name="sdt_t", tag="sdt_t")
                        nc.vector.tensor_tensor(
                            sdt_t[:], ps2[:], adr_sb[:, l, :], op=OP.mult
                        )
                        sdv_t = b1pool.tile([128, 4], dt.float32, name="sdv_t", tag="sdv_t")
                        nc.vector.reduce_sum(
                            sdv_t[:], sdt_t[:].rearrange("p (h c) -> p h c", c=C),
                            axis=AX.X,
                        )
                        sdrow = b1pool.tile([128, 128], dt.bfloat16, name="sdrow", tag="sdrow")
                        nc.vector.memset(sdrow[:], 0.0)
                        nc.vector.tensor_copy(sdrow[:, 0:4], sdv_t[:])
                        nc.sync.dma_start(s_dram[l][w * 128 : (w + 1) * 128, :], sdrow[:])

                    # --- B2: AllGather feature table ---
                    if STAGE < 2:
                        continue
                    nc.gpsimd.collective_compute(
                        "AllGather", OP.bypass, replica_groups=RG,
                        ins=[in_cc[l].opt()], outs=[out_cc[l].opt()],
                    )
                    if STAGE < 3:
                        continue

                    # --- B3: windows ---
                    nwin = NTILES if (STAGE >= 5 and not _sub) else 1
                    for w in range(nwin):
                        tl, th = T_lo[w], T_hi[w]
                        tw = tl + th
                        g = wpool.tile([128, TMAX, HID], dt.bfloat16, name="g", tag="g")
                        sd = wpool.tile([128, TMAX, 128], dt.bfloat16, name="sd", tag="sd")
                        en_glo = STAGE not in (21, 22, 23)
                        en_ghi = STAGE not in (20, 22, 23)
                        en_sd = STAGE not in (20, 21, 22)
                        if STAGE in (20, 21, 22, 23):
                            nc.vector.memset(g[:], 0.0)
                            nc.vector.memset(sd[:], 0.0)
                        if en_glo:
                            nc.gpsimd.dma_gather(
                                g[:, 0:tl, :],
                                out_cc[l][0:HALF, :],
                                gidx_sb[:, gcall[2 * w] : gcall[2 * w] + tl * 8],
                                num_idxs=tl * 128,
                                num_idxs_reg=tl * 128,
                                elem_size=HID,
                                single_packet=False,
                            )
                        if en_ghi:
                            nc.gpsimd.dma_gather(
                                g[:, tl:tw, :],
                                out_cc[l][HALF : 2 * HALF, :],
                                gidx_sb[:, gcall[2 * w + 1] : gcall[2 * w + 1] + th * 8],
                                num_idxs=th * 128,
                                num_idxs_reg=th * 128,
                                elem_size=HID,
                                single_packet=False,
                            )
                        if en_sd:
                            nc.gpsimd.dma_gather(
                                sd[:, 0:tw, :],
                                s_dram[l][:, :],
                                sdidx_sb[:, sdoff[w] : sdoff[w] + tw * 8],
                                num_idxs=tw * 128,
                                num_idxs_reg=tw * 128,
                                elem_size=128,
                                single_packet=False,
                            )
                        def consume(ap):
                            nc.vector.tensor_copy(h_sb[:, w, 0 : ap.shape[-1]], ap)

                        if STAGE in (20, 21, 22, 23):
                            consume(g[:, 0, 0:HID])
                            consume(sd[:, 0, 0:128])
                            continue
                        if STAGE == 30:
                            consume(g[:, 0, 0:HID])
                            continue
                        if STAGE == 31:
                            consume(g[:, 0, 0:HID])
                            consume(sd[:, 0, 0:128])
                            continue
                        if STAGE < 4:
                            continue
                        tmp2 = wpool2.tile([128, TMAX, HID], dt.bfloat16, name="tmp2", tag="tmp2")
                        nc.vector.tensor_tensor(
                            tmp2[:, 0:tw, :],
                            g[:, 0:tw, :],
                            asr_sb[:, l, :].unsqueeze(1).broadcast_to([128, tw, HID]),
                            op=OP.mult,
                        )
                        ssrc = wpool.tile([128, TMAX, 4], dt.float32, name="ssrc", tag="ssrc")
                        nc.vector.reduce_sum(
                            ssrc[:, 0:tw, :],
                            tmp2[:, 0:tw, :].rearrange("p t (h c) -> p t h c", c=C),
                            axis=AX.X,
                        )
                        if STAGE == 32:
                            consume(ssrc[:, 0, 0:4])
                            continue
                        ef = wpool.tile([128, TMAX, 4], dt.float32, name="ef", tag="ef")
                        nc.vector.tensor_tensor(
                            ef[:, 0:tw, :], ssrc[:, 0:tw, :], sd[:, 0:tw, 0:4], op=OP.add
                        )
                        e2 = wpool.tile([128, TMAX, 4], dt.float32, name="e2", tag="e2")
                        nc.vector.tensor_scalar_mul(e2[:, 0:tw, :], ef[:, 0:tw, :], NEG_SLOPE)
                        nc.vector.tensor_tensor(
                            e2[:, 0:tw, :], ef[:, 0:tw, :], e2[:, 0:tw, :], op=OP.max
                        )
                        ex = wpool.tile([128, TMAX, 4], dt.float32, name="ex", tag="ex")
                        nc.scalar.activation(
                            ex[:, 0:tw, :], e2[:, 0:tw, :], ACT.Exp, bias=zero1[:]
                        )
                        if STAGE == 33:
                            consume(ex[:, 0, 0:4])
                            continue
                        oh = wpool.tile([128, TMAX, 128], dt.bfloat16, name="oh", tag="oh")
                        nc.vector.tensor_tensor(
                            oh[:, 0:tw, :],
                            ids_sb[:, idoff[w] : idoff[w] + tw]
                            .unsqueeze(2).broadcast_to([128, tw, 128]),
                            iota_sb[:].unsqueeze(1).broadcast_to([128, tw, 128]),
                            op=OP.is_equal,
                        )
                        if STAGE == 34:
                            consume(oh[:, 0, 0:128])
                            continue
                        rhsw = wpool2.tile(
                            [128, TMAX, HID + 4], dt.bfloat16, name="rhsw", tag="rhsw"
                        )
                        for h in range(HEADS):
                            nc.vector.tensor_tensor(
                                rhsw[:, 0:tw, h * C : (h + 1) * C],
                                g[:, 0:tw, h * C : (h + 1) * C],
                                ex[:, 0:tw, h : h + 1].broadcast_to([128, tw, C]),
                                op=OP.mult,
                            )
                        nc.vector.tensor_copy(rhsw[:, 0:tw, HID : HID + 4], ex[:, 0:tw, :])
                        if STAGE == 35:
                            consume(rhsw[:, 0, 0:HID])
                            continue
                        psw = wps.tile([128, HID + 4], dt.float32, name="psw", tag="psw")
                        for t in range(tw):
                            nc.tensor.matmul(
                                psw[:], oh[:, t, :], rhsw[:, t, :],
                                start=(t == 0), stop=(t == tw - 1),
                            )
                        if STAGE == 36:
                            consume(psw[:, 0:HID])
                            continue
                        # epilogue
                        den = wpool.tile([128, 4], dt.float32, name="den", tag="den")
                        nc.vector.tensor_scalar_max(den[:], psw[:, HID : HID + 4], 1e-30)
                        rec = wpool.tile([128, 4], dt.float32, name="rec", tag="rec")
                        nc.vector.reciprocal(rec[:], den[:])
                        if STAGE == 37:
                            consume(rec[:])
                            continue
                        xn = wpool.tile([128, HID], dt.float32, name="xn", tag="xn")
                        nc.vector.tensor_tensor(
                            xn[:].rearrange("p (h c) -> p h c", c=C),
                            psw[:, 0:HID].rearrange("p (h c) -> p h c", c=C),
                            rec[:].unsqueeze(2).broadcast_to([128, 4, C]),
                            op=OP.mult,
                        )
                        nc.vector.tensor_tensor(xn[:], xn[:], gbr_sb[:, l, :], op=OP.add)
                        nc.vector.tensor_scalar_max(xn[:], xn[:], 0.0)
                        if STAGE == 38:
                            consume(xn[:])
                            continue
                        mus = wpool.tile([128, 1], dt.float32, name="mus", tag="mus")
                        nc.vector.reduce_sum(mus[:], xn[:], axis=AX.X)
                        mu = wpool.tile([128, 1], dt.float32, name="mu", tag="mu")
                        nc.scalar.mul(mu[:], mus[:], 1.0 / HID)
                        nc.vector.tensor_scalar_sub(xn[:], xn[:], mu[:])
                        if STAGE == 39:
                            consume(xn[:])
                            continue
                        sq = wpool.tile([128, HID], dt.float32, name="sq", tag="sq")
                        nc.vector.tensor_tensor(sq[:], xn[:], xn[:], op=OP.mult)
                        nc.vector.reduce_sum(var_sb[:, w : w + 1], sq[:], axis=AX.X)
                        nc.vector.tensor_tensor(
                            h_sb[:, w, :], xn[:], lgr_sb[:, l, :], op=OP.mult
                        )

                    # --- batched rsqrt + fixup + bias ---
                    if STAGE < 5 or _sub:
                        continue
                    sdv = b1pool.tile([128, NTILES], dt.float32, name="sdv", tag="sdv")
                    nc.scalar.activation(
                        sdv[:], var_sb[:], ACT.Sqrt, bias=eps1[:], scale=1.0 / HID
                    )
                    nc.vector.reciprocal(rstd_sb[:], sdv[:])
                    nc.vector.tensor_tensor(
                        h_sb[:], h_sb[:],
                        rstd_sb[:].unsqueeze(2).broadcast_to([128, NTILES, HID]),
                        op=OP.mult,
                    )
                    nc.vector.tensor_tensor(
                        h_sb[:], h_sb[:],
                        lbr_sb[:, l, :].unsqueeze(1).broadcast_to([128, NTILES, HID]),
                        op=OP.add,
                    )

                    # --- transposes for next layer ---
                    if l < L - 1:
                        for w in range(NTILES):
                            for k in range(2):
                                pst = tps.tile([128, 128], dt.bfloat16, name="pst", tag="pst")
                                nc.tensor.transpose(
                                    pst[:], h_sb[:, w, k * 128 : (k + 1) * 128], identb_sb[:]
                                )
                                nc.vector.tensor_copy(
                                    hT[:, k * PADSHARD + w * 128 : k * PADSHARD + (w + 1) * 128],
                                    pst[:],
                                )

            # ---- pooling + readout ----
            if STAGE < 99 and not STAGE == 7:
                with tc.tile_pool(name="dbg", bufs=1) as dbgp:
                    dbg = dbgp.tile([B, HID], dt.float32, name="dbg")
                    if STAGE >= 30:
                        nc.vector.tensor_copy(dbg[:], h_sb[0:B, 0, :])
                    else:
                        nc.vector.tensor_copy(dbg[:], hT[0:B, 0:HID])
                    nc.sync.dma_start(t_out.ap(), dbg[:])
            else:
              with (
                tc.tile_pool(name="ro", bufs=1) as ropool,
                tc.tile_pool(name="rops", bufs=1, space="PSUM") as rops,
              ):
                psp = rops.tile([B, HID], dt.float32, name="psp")
                for w in range(NTILES):
                    nc.tensor.matmul(
                        psp[:], ohb_sb[:, w, :], h_sb[:, w, :],
                        start=(w == 0), stop=(w == NTILES - 1),
                    )
                pp = ropool.tile([B, HID], dt.float32, name="pp")
                nc.vector.tensor_copy(pp[:], psp[:])
                nc.sync.dma_start(ar_in[:], pp[:])
                nc.gpsimd.collective_compute(
                    "AllReduce", OP.add, replica_groups=RG,
                    ins=[ar_in.opt()], outs=[ar_out.opt()],
                )
                pooled = ropool.tile([B, HID], dt.float32, name="pooled")
                nc.sync.dma_start(pooled[:], ar_out[:])
                nc.vector.tensor_scalar_mul(pooled[:], pooled[:], invc_sb[:])

                pldT = ropool.tile([128, 2, B], dt.float32, name="pldT")
                for k in range(2):
                    pstf = rops.tile([128, B], dt.float32, name="pstf", tag="pstf")
                    nc.tensor.transpose(
                        pstf[:], pooled[:, k * 128 : (k + 1) * 128], identf_sb[0:B, 0:B]
                    )
                    nc.vector.tensor_copy(pldT[:, k, :], pstf[:])
                z1ps = rops.tile([B, HID], dt.float32, name="z1ps")
                for k in range(2):
                    nc.tensor.matmul(
                        z1ps[:], pldT[:, k, :], row1_sb[:, k, :],
                        start=(k == 0), stop=(k == 1),
                    )
                z1b = ropool.tile([B, HID], dt.float32, name="z1b")
                nc.vector.tensor_tensor(z1b[:], z1ps[:], b1r_sb[:], op=OP.add)
                z1g = ropool.tile([B, HID], dt.float32, name="z1g")
                gelu_fn = ACT.Identity if SIM_MODE else ACT.Gelu
                nc.scalar.activation(z1g[:], z1b[:], gelu_fn, bias=zero1[0:B, :])
                z1gT = ropool.tile([128, 2, B], dt.float32, name="z1gT")
                for k in range(2):
                    pstf2 = rops.tile([128, B], dt.float32, name="pstf2", tag="pstf")
                    nc.tensor.transpose(
                        pstf2[:], z1g[:, k * 128 : (k + 1) * 128], identf_sb[0:B, 0:B]
                    )
                    nc.vector.tensor_copy(z1gT[:, k, :], pstf2[:])
                z2ps = rops.tile([B, HID], dt.float32, name="z2ps")
                for k in range(2):
                    nc.tensor.matmul(
                        z2ps[:], z1gT[:, k, :], row2_sb[:, k, :],
                        start=(k == 0), stop=(k == 1),
                    )
                ob = ropool.tile([B, HID], dt.float32, name="ob")
                nc.vector.tensor_tensor(ob[:], z2ps[:], b2r_sb[:], op=OP.add)
                nc.sync.dma_start(t_out.ap(), ob[:])

    nc.compile()
    return nc


last_exec_ns = None
last_results = None


def _pjrt_timed(nc, in_maps, iters=30):
    """Execute the compiled Bass graph via PJRT with device-resident inputs,
    returning (out_core0, per-iter wall times)."""
    import time

    import jax
    from jax.experimental.shard_map import shard_map
    from jax.sharding import Mesh, NamedSharding, PartitionSpec

    from concourse import bass2jax, mybir

    bass2jax.install_neuronx_cc_hook()

    partition_name = nc.partition_id_tensor.name if nc.partition_id_tensor else None
    in_names, out_names, out_avals, zero_outs = [], [], [], []
    for alloc in nc.m.functions[0].allocations:
        if not isinstance(alloc, mybir.MemoryLocationSet):
            continue
        name = alloc.memorylocations[0].name
        if alloc.kind == "ExternalInput":
            if name != partition_name:
                in_names.append(name)
        elif alloc.kind == "ExternalOutput":
            out_names.append(name)
            shape = tuple(alloc.tensor_shape)
            dtype = mybir.dt.np(alloc.dtype)
            out_avals.append(jax.core.ShapedArray(shape, dtype))
            zero_outs.append(np.zeros(shape, dtype))
    n_params = len(in_names)
    all_names = list(in_names) + list(out_names)
    if partition_name is not None:
        all_names.append(partition_name)
    all_names = tuple(all_names)

    def _body(*args):
        operands = list(args)
        if partition_name is not None:
            operands.append(bass2jax.partition_id_tensor())
        outs = bass2jax._bass_exec_p.bind(
            *operands,
            out_avals=tuple(out_avals),
            in_names=all_names,
            out_names=tuple(out_names),
            lowering_input_output_aliases=(),
            sim_require_finite=True,
            sim_require_nnan=True,
            nc=nc,
        )
        return tuple(outs)

    ncores = len(in_maps)
    devices = jax.devices()[:ncores]
    mesh = Mesh(np.asarray(devices), ("core",))
    in_specs = (PartitionSpec("core"),) * (n_params + len(out_names))
    out_specs = (PartitionSpec("core"),) * len(out_names)
    fn = jax.jit(
        shard_map(_body, mesh=mesh, in_specs=in_specs, out_specs=out_specs,
                  check_rep=False),
        keep_unused=True,
    )
    sh = NamedSharding(mesh, PartitionSpec("core"))
    dev_in = [
        jax.device_put(
            np.concatenate([np.asarray(in_maps[c][k]) for c in range(ncores)], axis=0), sh
        )
        for k in in_names
    ] + [
        jax.device_put(np.zeros((ncores * z.shape[0], *z.shape[1:]), z.dtype), sh)
        for z in zero_outs
    ]
    r = fn(*dev_in)
    jax.block_until_ready(r)
    times = []
    for _ in range(iters):
        t0 = time.perf_counter()
        r = fn(*dev_in)
        jax.block_until_ready(r)
        times.append(time.perf_counter() - t0)
    out = np.asarray(r[0]).reshape(ncores, *out_avals[0].shape)[0]
    return out, times


def bench(inputs, iters=30):
    """Returns (out, times_list). Uses cached compiled graph."""
    in_maps, struct = _prep(inputs)
    key = (tuple(struct["T_lo"]), tuple(struct["T_hi"]))
    if key not in _cache:
        _cache[key] = _build(struct)
    return _pjrt_timed(_cache[key], in_maps, iters=iters)


def kernel(**inputs):
    global last_exec_ns, last_results
    from concourse import bass_utils

    in_maps, struct = _prep(inputs)
    key = (tuple(struct["T_lo"]), tuple(struct["T_hi"]))
    if key not in _cache:
        _cache[key] = _build(struct)
    nc = _cache[key]

    trace = os.environ.get("BASS_GNN_TRACE", "0") == "1"
    res = bass_utils.run_bass_kernel_spmd(
        nc, in_maps, core_ids=list(range(NCORES)), trace=trace
    )
    last_exec_ns = res.exec_time_ns
    last_results = res
    return np.asarray(res.results[0]["out"], np.float32)


# revision 49
# speedup vs baseline: 19.8531x; 19.8531x over previous
"""Trainium2 Bass kernel for AIRS-GNN (4-layer GAT + readout) on 8 NeuronCores.

Self-contained: hardcodes all shapes/sharding. Host side does only integer
index manipulation (edge partitioning, padding, one-hot layout) plus constant
table construction; all floating-point math runs on device.

Sharding: nodes are sharded contiguously across the 8 cores (6250/core,
padded to 6272 = 49*128 rows).  Edges are owned by the core of their dst
node (pull-based aggregation).  Per layer the bf16 node-feature table
(h @ W) is AllGathered, then each core gathers the rows for its edges with
dma_gather and scatter-adds messages into per-window PSUM via one-hot
matmuls.  Softmax normalization is applied after aggregation (denominators
ride along as 4 extra matmul columns); the segment-max shift of the
reference cancels algebraically and is skipped.
"""

import os
import numpy as np
import ml_dtypes

# ---------------- problem constants (from spec) ----------------
N, E, F, HID, HEADS, L, B, R = 50000, 400000, 64, 256, 4, 4, 16, 8
C = HID // HEADS  # 64
NCORES = 8
SHARD = N // NCORES            # 6250
NTILES = (SHARD + 127) // 128  # 49
PADSHARD = NTILES * 128        # 6272
HALF = PADSHARD * (NCORES // 2)  # 25088 rows per half-table
LN_EPS = 1e-5
NEG_SLOPE = 0.2

BF16 = ml_dtypes.bfloat16

SIM_MODE = False  # debug: replace sim-unsupported Gelu with Identity
STAGE = 99  # debug: truncate graph after this stage (99 = full)

_cache = {}


def _posenc(n, d):
    pos = np.arange(n, dtype=np.float32)[:, None]
    i = np.arange(d, dtype=np.float32)[None, :]
    rates = (1.0 / 10000.0 ** (2.0 * np.floor(i / 2.0) / d)).astype(np.float32)
    ang = pos * rates
    return np.where(np.arange(d)[None, :] % 2 == 0, np.sin(ang), np.cos(ang)).astype(
        np.float32
    )


def _wrap16(a):
    """dma_gather index layout: idxs[p, s] = a[s*16 + p], replicated to 128 parts."""
    assert a.size % 16 == 0
    w = a.reshape(-1, 16).T.astype(np.int16)  # [16, S/16]
    return np.tile(w, (8, 1))  # [128, S/16]


def _prep(inputs):
    """Host-side integer prep. Returns (in_maps, struct)."""
    ei = np.asarray(inputs["edge_index"])
    src = np.concatenate([np.asarray(ei[0]), np.arange(N)]).astype(np.int64)
    dst = np.concatenate([np.asarray(ei[1]), np.arange(N)]).astype(np.int64)

    core = dst // SHARD
    dloc = dst - core * SHARD
    win = dloc // 128
    drel = dloc - win * 128
    grow = (src // SHARD) * PADSHARD + (src % SHARD)  # padded global table row
    half = (grow >= HALF).astype(np.int64)

    # tiles needed per (window, half): max over cores
    cnt = np.zeros((NCORES, NTILES, 2), np.int64)
    np.add.at(cnt, (core, win, half), 1)
    T = np.maximum((cnt.max(axis=0) + 127) // 128, 1)  # [NTILES, 2]
    T_lo, T_hi = T[:, 0], T[:, 1]
    Tw = T_lo + T_hi
    wslot0 = np.concatenate([[0], np.cumsum(Tw * 128)])
    TOTSLOT = int(wslot0[-1])

    gidx_maps, sdidx_maps, ids_maps = [], [], []
    for c in range(NCORES):
        sel = core == c
        wc, dc, gc, hc = win[sel], drel[sel], grow[sel], half[sel]
        key = wc * 2 + hc
        order = np.argsort(key, kind="stable")
        wc, dc, gc, hc = wc[order], dc[order], gc[order], hc[order]
        ks = key[order]
        grp_start = np.searchsorted(ks, np.arange(NTILES * 2), side="left")
        pos = np.arange(ks.size) - grp_start[ks]
        slot = wslot0[wc] + np.where(hc == 1, T_lo[wc] * 128, 0) + pos
        assert slot.max() < TOTSLOT

        g_val = np.zeros(TOTSLOT, np.int64)          # pad -> row 0 (valid)
        g_val[slot] = gc - hc * HALF
        sd_val = np.zeros(TOTSLOT, np.int64)
        sd_val[slot] = wc * 128 + dc
        id_val = np.full(TOTSLOT, -1.0, np.float32)  # pad -> -1 (no one-hot match)
        id_val[slot] = dc

        gcols, sdcols, idcols = [], [], []
        for w in range(NTILES):
            s0 = wslot0[w]
            nlo, nhi = int(T_lo[w]) * 128, int(T_hi[w]) * 128
            gcols.append(_wrap16(g_val[s0 : s0 + nlo]))
            gcols.append(_wrap16(g_val[s0 + nlo : s0 + nlo + nhi]))
            sdcols.append(_wrap16(sd_val[s0 : s0 + nlo + nhi]))
            idcols.append(id_val[s0 : s0 + nlo + nhi].reshape(-1, 128).T.astype(BF16))
        gidx_maps.append(np.concatenate(gcols, axis=1))
        sdidx_maps.append(np.concatenate(sdcols, axis=1))
        ids_maps.append(np.concatenate(idcols, axis=1))

    # ---------------- dense/static per-core tensors ----------------
    x = np.asarray(inputs["x"], np.float32)
    region_ids = np.asarray(inputs["region_ids"]).astype(np.int64)
    batch = np.asarray(inputs["batch"]).astype(np.int64)
    pe = _posenc(N, F)

    counts = np.bincount(batch, minlength=B).astype(np.float32)
    inv_cnt = (1.0 / np.maximum(counts, 1.0)).astype(np.float32)[:, None]

    w_in = np.asarray(inputs["in_proj_w"], np.float32)  # [3F, HID]
    gat_w = np.asarray(inputs["gat_w"], np.float32)     # [L, HID, HID]
    # [128, L*2, HID]: chunk (l,k) at [:, l*2+k, :]
    gatw_h = np.ascontiguousarray(
        gat_w.reshape(L, 2, 128, HID).transpose(2, 0, 1, 3).reshape(128, L * 2, HID)
    ).astype(BF16)

    def rep128(a, d):  # [L, HID] -> [128, L, HID] replicated, dtype d
        return np.ascontiguousarray(
            np.broadcast_to(np.asarray(a, np.float32)[None, :, :], (128, L, HID))
        ).astype(d)

    asr_h = rep128(np.asarray(inputs["att_src"], np.float32).reshape(L, HID), BF16)
    adr_h = rep128(np.asarray(inputs["att_dst"], np.float32).reshape(L, HID), BF16)
    gbr_h = rep128(inputs["gat_b"], np.float32)
    lgr_h = rep128(inputs["ln_g"], np.float32)
    lbr_h = rep128(inputs["ln_b"], BF16)
    ipb_h = np.ascontiguousarray(
        np.asarray(inputs["in_proj_b"], np.float32).reshape(2, 128).T
    )  # [128, 2]
    row1_h = np.ascontiguousarray(
        np.asarray(inputs["ro_w1"], np.float32).reshape(2, 128, HID).transpose(1, 0, 2)
    )  # [128, 2, HID]
    row2_h = np.ascontiguousarray(
        np.asarray(inputs["ro_w2"], np.float32).reshape(2, 128, HID).transpose(1, 0, 2)
    )

    in_maps = []
    for c in range(NCORES):
        lo, hi = c * SHARD, (c + 1) * SHARD
        xT = np.zeros((F, PADSHARD), np.float32)
        xT[:, :SHARD] = x[lo:hi].T
        peT = np.zeros((F, PADSHARD), np.float32)
        peT[:, :SHARD] = pe[lo:hi].T
        rT = np.zeros((R, PADSHARD), np.float32)
        rT[region_ids[lo:hi], np.arange(SHARD)] = 1.0
        ohb = np.zeros((128, NTILES, B), BF16)
        p_all = np.arange(SHARD)
        ohb[p_all % 128, p_all // 128, batch[lo:hi]] = 1.0

        m = {
            "xT": xT,
            "peT": peT,
            "rT": rT,
            "gidx": gidx_maps[c].astype(np.int16),
            "sdidx": sdidx_maps[c].astype(np.int16),
            "ids": ids_maps[c].astype(BF16),
            "ohb": ohb,
            "w_x": w_in[:F].copy(),
            "w_r2": w_in[F : 2 * F].copy(),
            "w_p": w_in[2 * F :].copy(),
            "embT": np.asarray(inputs["region_emb_w"], np.float32).T.copy(),
            "ipb": ipb_h,
            "gatw": gatw_h,
            "asr": asr_h,
            "adr": adr_h,
            "gbr": gbr_h,
            "lgr": lgr_h,
            "lbr": lbr_h,
            "row1": row1_h,
            "row2": row2_h,
            "b1r": np.ascontiguousarray(
                np.broadcast_to(np.asarray(inputs["ro_b1"], np.float32)[None, :], (B, HID))
            ),
            "b2r": np.ascontiguousarray(
                np.broadcast_to(np.asarray(inputs["ro_b2"], np.float32)[None, :], (B, HID))
            ),
            "invc": inv_cnt,
            "iota": np.ascontiguousarray(
                np.broadcast_to(np.arange(128, dtype=np.float32)[None, :], (128, 128))
            ).astype(BF16),
            "identb": np.eye(128, dtype=np.float32).astype(BF16),
            "identf": np.eye(128, dtype=np.float32),
        }
        in_maps.append(m)

    struct = {
        "T_lo": [int(t) for t in T_lo],
        "T_hi": [int(t) for t in T_hi],
        "GCOLS": int(gidx_maps[0].shape[1]),
        "SDCOLS": int(sdidx_maps[0].shape[1]),
        "IDCOLS": int(ids_maps[0].shape[1]),
    }
    return in_maps, struct


def _build(struct):
    """Build the Bass graph (identical for all cores)."""
    import concourse.bass as bass  # noqa: F401
    import concourse.tile as tile
    from concourse import bacc, mybir

    dt = mybir.dt
    AX = mybir.AxisListType
    OP = mybir.AluOpType
    ACT = mybir.ActivationFunctionType

    T_lo, T_hi = struct["T_lo"], struct["T_hi"]
    Tw = [a + b for a, b in zip(T_lo, T_hi)]
    TMAX = max(Tw)

    nc = bacc.Bacc(
        "TRN2", target_bir_lowering=False, debug=False,
        num_devices=(1 if STAGE == 98 else NCORES),
    )
    RG = [list(range(NCORES))]

    def din(name, shape, d=dt.float32):
        return nc.dram_tensor(name, shape, d, kind="ExternalInput")

    t_xT = din("xT", [F, PADSHARD])
    t_peT = din("peT", [F, PADSHARD])
    t_rT = din("rT", [R, PADSHARD])
    t_gidx = din("gidx", [128, struct["GCOLS"]], dt.int16)
    t_sdidx = din("sdidx", [128, struct["SDCOLS"]], dt.int16)
    t_ids = din("ids", [128, struct["IDCOLS"]], dt.bfloat16)
    t_ohb = din("ohb", [128, NTILES, B], dt.bfloat16)
    t_wx = din("w_x", [F, HID])
    t_wr2 = din("w_r2", [F, HID])
    t_wp = din("w_p", [F, HID])
    t_embT = din("embT", [F, R])
    t_ipb = din("ipb", [128, 2])
    t_gatw = din("gatw", [128, L * 2, HID], dt.bfloat16)
    t_asr = din("asr", [128, L, HID], dt.bfloat16)
    t_adr = din("adr", [128, L, HID], dt.bfloat16)
    t_gbr = din("gbr", [128, L, HID])
    t_lgr = din("lgr", [128, L, HID])
    t_lbr = din("lbr", [128, L, HID], dt.bfloat16)
    t_row1 = din("row1", [128, 2, HID])
    t_row2 = din("row2", [128, 2, HID])
    t_b1r = din("b1r", [B, HID])
    t_b2r = din("b2r", [B, HID])
    t_invc = din("invc", [B, 1])
    t_iota = din("iota", [128, 128], dt.bfloat16)
    t_identb = din("identb", [128, 128], dt.bfloat16)
    t_identf = din("identf", [128, 128])

    t_out = nc.dram_tensor("out", [B, HID], dt.float32, kind="ExternalOutput")

    # static per-window offsets (in idx columns / id columns)
    gcall = [0]
    for w in range(NTILES):
        gcall.append(gcall[-1] + T_lo[w] * 8)
        gcall.append(gcall[-1] + T_hi[w] * 8)
    sdoff = [0]
    idoff = [0]
    for w in range(NTILES):
        sdoff.append(sdoff[-1] + Tw[w] * 8)
        idoff.append(idoff[-1] + Tw[w])

    with tile.TileContext(nc) as tc:
        with (
            tc.tile_pool(name="const", bufs=1) as cpool,
            tc.tile_pool(name="dram", bufs=1, space="DRAM") as dpool,
            tc.tile_pool(name="persist", bufs=1) as ppool,
        ):
            def load(t, shape, d=dt.float32):
                tl = cpool.tile(shape, d, name=t.name + "_sb")
                nc.sync.dma_start(tl[:], t.ap())
                return tl

            gidx_sb = load(t_gidx, [128, struct["GCOLS"]], dt.int16)
            sdidx_sb = load(t_sdidx, [128, struct["SDCOLS"]], dt.int16)
            ids_sb = load(t_ids, [128, struct["IDCOLS"]], dt.bfloat16)
            ohb_sb = load(t_ohb, [128, NTILES, B], dt.bfloat16)
            ipb_sb = load(t_ipb, [128, 2])
            gatw_sb = load(t_gatw, [128, L * 2, HID], dt.bfloat16)
            asr_sb = load(t_asr, [128, L, HID], dt.bfloat16)
            adr_sb = load(t_adr, [128, L, HID], dt.bfloat16)
            gbr_sb = load(t_gbr, [128, L, HID])
            lgr_sb = load(t_lgr, [128, L, HID])
            lbr_sb = load(t_lbr, [128, L, HID], dt.bfloat16)
            row1_sb = load(t_row1, [128, 2, HID])
            row2_sb = load(t_row2, [128, 2, HID])
            b1r_sb = load(t_b1r, [B, HID])
            b2r_sb = load(t_b2r, [B, HID])
            invc_sb = load(t_invc, [B, 1])
            iota_sb = load(t_iota, [128, 128], dt.bfloat16)
            identb_sb = load(t_identb, [128, 128], dt.bfloat16)
            identf_sb = load(t_identf, [128, 128])

            zero1 = cpool.tile([128, 1], dt.float32, name="zero1")
            nc.vector.memset(zero1[:], 0.0)
            eps1 = cpool.tile([128, 1], dt.float32, name="eps1")
            nc.vector.memset(eps1[:], LN_EPS)

            hT = ppool.tile([128, 2 * PADSHARD], dt.bfloat16, name="hT")
            h_sb = ppool.tile([128, NTILES, HID], dt.bfloat16, name="h_sb")
            var_sb = ppool.tile([128, NTILES], dt.float32, name="var_sb")
            rstd_sb = ppool.tile([128, NTILES], dt.float32, name="rstd_sb")

            GW = HID + 128  # 384-col rows: [h2 | s_src | pad], 768B
            in_cc = [
                dpool.tile([PADSHARD, GW], dt.bfloat16, name=f"incc{l}")
                for l in range(L)
            ]
            out_cc = [
                dpool.tile(
                    [NCORES * PADSHARD, GW], dt.bfloat16, name=f"outcc{l}",
                    addr_space="Shared",
                )
                for l in range(L)
            ]
            s_dram = [
                dpool.tile([PADSHARD, 128], dt.bfloat16, name=f"sdram{l}")
                for l in range(L)
            ]
            ar_in = dpool.tile([B, HID], dt.float32, name="ar_in")
            ar_out = dpool.tile([B, HID], dt.float32, name="ar_out", addr_space="Shared")

            # ---- stage A: input projection -> hT (bf16) ----
            with (
                tc.tile_pool(name="aproj", bufs=1) as apool,
                tc.tile_pool(name="apsum", bufs=2, space="PSUM") as appool,
            ):
                xT_sb = apool.tile([F, PADSHARD], dt.float32, name="xT_sb")
                nc.sync.dma_start(xT_sb[:], t_xT.ap())
                peT_sb = apool.tile([F, PADSHARD], dt.float32, name="peT_sb")
                nc.sync.dma_start(peT_sb[:], t_peT.ap())
                rT_sb = apool.tile([R, PADSHARD], dt.float32, name="rT_sb")
                nc.sync.dma_start(rT_sb[:], t_rT.ap())
                wx_sb = apool.tile([F, HID], dt.float32, name="wx_sb")
                nc.sync.dma_start(wx_sb[:], t_wx.ap())
                wr2_sb = apool.tile([F, HID], dt.float32, name="wr2_sb")
                nc.sync.dma_start(wr2_sb[:], t_wr2.ap())
                wp_sb = apool.tile([F, HID], dt.float32, name="wp_sb")
                nc.sync.dma_start(wp_sb[:], t_wp.ap())
                embT_sb = apool.tile([F, R], dt.float32, name="embT_sb")
                nc.sync.dma_start(embT_sb[:], t_embT.ap())

                ew_ps = appool.tile([R, HID], dt.float32, name="ew_ps")
                nc.tensor.matmul(ew_ps[:], embT_sb[:], wr2_sb[:])
                ew_sb = apool.tile([R, HID], dt.float32, name="ew_sb")
                nc.vector.tensor_copy(ew_sb[:], ew_ps[:])

                NBLK = 512
                nblocks = (PADSHARD + NBLK - 1) // NBLK
                for k in range(2):
                    fs = slice(k * 128, (k + 1) * 128)
                    for nb in range(nblocks):
                        c0 = nb * NBLK
                        cw = min(NBLK, PADSHARD - c0)
                        ps = appool.tile([128, NBLK], dt.float32, name="aps", tag="aps")
                        nc.tensor.matmul(
                            ps[:, :cw], wx_sb[:, fs], xT_sb[:, c0 : c0 + cw],
                            start=True, stop=False,
                        )
                        nc.tensor.matmul(
                            ps[:, :cw], ew_sb[:, fs], rT_sb[:, c0 : c0 + cw],
                            start=False, stop=False,
                        )
                        nc.tensor.matmul(
                            ps[:, :cw], wp_sb[:, fs], peT_sb[:, c0 : c0 + cw],
                            start=False, stop=True,
                        )
                        nc.vector.tensor_scalar_add(
                            hT[:, k * PADSHARD + c0 : k * PADSHARD + c0 + cw],
                            ps[:, :cw],
                            ipb_sb[:, k : k + 1],
                        )

            # ---- GAT layers ----
            with (
                tc.tile_pool(name="b1", bufs=3) as b1pool,
                tc.tile_pool(name="b1ps", bufs=2, space="PSUM") as b1ps,
                tc.tile_pool(name="win", bufs=4) as wpool,
                tc.tile_pool(name="win2", bufs=2) as wpool2,
                tc.tile_pool(name="wps", bufs=3, space="PSUM") as wps,
                tc.tile_pool(name="tps", bufs=2, space="PSUM") as tps,
            ):
                _sub = 20 <= STAGE < 40
                if STAGE in (50, 51, 52, 98) or 60 <= STAGE <= 66:
                    nlayers = L
                elif STAGE < 1:
                    nlayers = 0
                elif STAGE < 6 or _sub:
                    nlayers = 1
                else:
                    nlayers = L
                for l in range(nlayers):
                    # --- B1: h2 = h @ W_l; s_dst; feature table ---
                    for w in range(NTILES):
                        ps2 = b1ps.tile([128, HID], dt.float32, name="h2ps", tag="h2ps")
                        for k in range(2):
                            nc.tensor.matmul(
                                ps2[:],
                                hT[:, k * PADSHARD + w * 128 : k * PADSHARD + (w + 1) * 128],
                                gatw_sb[:, l * 2 + k, :],
                                start=(k == 0),
                                stop=(k == 1),
                            )
                        g_t = b1pool.tile([128, GW], dt.bfloat16, name="g_t", tag="g_t")
                        if SIM_MODE:
                            nc.vector.memset(g_t[:], 0.0)
                        nc.vector.tensor_copy(g_t[:, 0:HID], ps2[:])
                        sst_t = b1pool.tile([128, HID], dt.bfloat16, name="sst_t", tag="sst_t")
                        nc.any.tensor_tensor(
                            sst_t[:], ps2[:], asr_sb[:, l, :], op=OP.mult
                        )
                        ssv_t = b1pool.tile([128, 4], dt.float32, name="ssv_t", tag="ssv_t")
                        nc.vector.reduce_sum(
                            ssv_t[:], sst_t[:].rearrange("p (h c) -> p h c", c=C),
                            axis=AX.X,
                        )
                        nc.vector.tensor_copy(g_t[:, HID : HID + 4], ssv_t[:])
                        nc.sync.dma_start(in_cc[l][w * 128 : (w + 1) * 128, :], g_t[:])
                        sdt_t = b1pool.tile([128, HID], dt.bfloat16, name="sdt_t", tag="sdt_t")
                        nc.any.tensor_tensor(
                            sdt_t[:], ps2[:], adr_sb[:, l, :], op=OP.mult
                        )
                        sdv_t = b1pool.tile([128, 4], dt.float32, name="sdv_t", tag="sdv_t")
                        nc.vector.reduce_sum(
                            sdv_t[:], sdt_t[:].rearrange("p (h c) -> p h c", c=C),
                            axis=AX.X,
                        )
                        sdrow = b1pool.tile([128, 128], dt.bfloat16, name="sdrow", tag="sdrow")
                        if SIM_MODE:
                            nc.vector.memset(sdrow[:], 0.0)
                        nc.vector.tensor_copy(sdrow[:, 0:4], sdv_t[:])
                        nc.sync.dma_start(s_dram[l][w * 128 : (w + 1) * 128, :], sdrow[:])

                    # --- B2: AllGather feature table ---
                    if STAGE < 2 or STAGE == 51:
                        continue
                    if STAGE == 98:
                        nc.sync.dma_start(out_cc[l][0:PADSHARD, :], in_cc[l][:, :])
                    else:
                        nc.gpsimd.collective_compute(
                            "AllGather", OP.bypass, replica_groups=RG,
                            ins=[in_cc[l].opt()], outs=[out_cc[l].opt()],
                        )
                    if STAGE < 3 or STAGE == 50:
                        continue

                    # --- B3: windows ---
                    nwin = NTILES if ((STAGE >= 5 and not _sub) or STAGE in (52, 98) or 60 <= STAGE <= 66) else 1
                    for w in range(nwin):
                        tl, th = T_lo[w], T_hi[w]
                        tw = tl + th
                        g = wpool.tile([128, TMAX, GW], dt.bfloat16, name="g", tag="g")
                        sd = wpool.tile([128, TMAX, 128], dt.bfloat16, name="sd", tag="sd")
                        en_glo = STAGE not in (21, 22, 23)
                        en_ghi = STAGE not in (20, 22, 23)
                        en_sd = STAGE not in (20, 21, 22)
                        if STAGE in (20, 21, 22, 23):
                            nc.vector.memset(g[:], 0.0)
                            nc.vector.memset(sd[:], 0.0)
                        if en_glo:
                            nc.gpsimd.dma_gather(
                                g[:, 0:tl, :],
                                out_cc[l][0:HALF, :],
                                gidx_sb[:, gcall[2 * w] : gcall[2 * w] + tl * 8],
                                num_idxs=tl * 128,
                                num_idxs_reg=tl * 128,
                                elem_size=GW,
                                single_packet=False,
                            )
                        if en_ghi:
                            nc.gpsimd.dma_gather(
                                g[:, tl:tw, :],
                                out_cc[l][HALF : 2 * HALF, :],
                                gidx_sb[:, gcall[2 * w + 1] : gcall[2 * w + 1] + th * 8],
                                num_idxs=th * 128,
                                num_idxs_reg=th * 128,
                                elem_size=GW,
                                single_packet=False,
                            )
                        if en_sd:
                            nc.gpsimd.dma_gather(
                                sd[:, 0:tw, :],
                                s_dram[l][:, :],
                                sdidx_sb[:, sdoff[w] : sdoff[w] + tw * 8],
                                num_idxs=tw * 128,
                                num_idxs_reg=tw * 128,
                                elem_size=128,
                                single_packet=False,
                            )
                        def consume(ap):
                            nc.vector.tensor_copy(h_sb[:, w, 0 : ap.shape[-1]], ap)

                        if STAGE in (20, 21, 22, 23, 52):
                            consume(g[:, 0, 0:HID])
                            consume(sd[:, 0, 0:128])
                            continue
                        if STAGE == 30:
                            consume(g[:, 0, 0:HID])
                            continue
                        if STAGE == 31:
                            consume(g[:, 0, 0:HID])
                            consume(sd[:, 0, 0:128])
                            continue
                        if STAGE < 4:
                            continue
                        if STAGE in (32, 60):
                            consume(g[:, 0, HID : HID + 4])
                            continue
                        ef = wpool.tile([128, TMAX, 4], dt.float32, name="ef", tag="ef")
                        nc.vector.tensor_tensor(
                            ef[:, 0:tw, :], g[:, 0:tw, HID : HID + 4],
                            sd[:, 0:tw, 0:4], op=OP.add
                        )
                        e2 = wpool.tile([128, TMAX, 4], dt.float32, name="e2", tag="e2")
                        nc.vector.tensor_scalar_mul(e2[:, 0:tw, :], ef[:, 0:tw, :], NEG_SLOPE)
                        nc.vector.tensor_tensor(
                            e2[:, 0:tw, :], ef[:, 0:tw, :], e2[:, 0:tw, :], op=OP.max
                        )
                        ex = wpool.tile([128, TMAX, 4], dt.float32, name="ex", tag="ex")
                        nc.scalar.activation(
                            ex[:, 0:tw, :], e2[:, 0:tw, :], ACT.Exp, bias=zero1[:]
                        )
                        if STAGE in (33, 61):
                            consume(ex[:, 0, 0:4])
                            continue
                        oh = wpool.tile([128, TMAX, 128], dt.bfloat16, name="oh", tag="oh")
                        nc.any.tensor_tensor(
                            oh[:, 0:tw, :],
                            ids_sb[:, idoff[w] : idoff[w] + tw]
                            .unsqueeze(2).broadcast_to([128, tw, 128]),
                            iota_sb[:].unsqueeze(1).broadcast_to([128, tw, 128]),
                            op=OP.is_equal,
                        )
                        if STAGE in (34, 62):
                            consume(oh[:, 0, 0:128])
                            continue
                        rhsw = wpool2.tile(
                            [128, TMAX, HID + 4], dt.bfloat16, name="rhsw", tag="rhsw"
                        )
                        nc.any.tensor_tensor(
                            rhsw[:, 0:tw, 0:HID].rearrange("p t (h c) -> p t h c", c=C),
                            g[:, 0:tw, 0:HID].rearrange("p t (h c) -> p t h c", c=C),
                            ex[:, 0:tw, :].unsqueeze(3).broadcast_to([128, tw, HEADS, C]),
                            op=OP.mult,
                        )
                        nc.any.tensor_copy(rhsw[:, 0:tw, HID : HID + 4], ex[:, 0:tw, :])
                        if STAGE in (35, 63):
                            consume(rhsw[:, 0, 0:HID])
                            continue
                        psw = wps.tile([128, HID + 4], dt.float32, name="psw", tag="psw")
                        for t in range(tw):
                            nc.tensor.matmul(
                                psw[:], oh[:, t, :], rhsw[:, t, :],
                                start=(t == 0), stop=(t == tw - 1),
                            )
                        if STAGE in (36, 64):
                            consume(psw[:, 0:HID])
                            continue
                        # epilogue
                        den = wpool.tile([128, 4], dt.float32, name="den", tag="den")
                        nc.vector.tensor_scalar_max(den[:], psw[:, HID : HID + 4], 1e-30)
                        rec = wpool.tile([128, 4], dt.float32, name="rec", tag="rec")
                        nc.vector.reciprocal(rec[:], den[:])
                        if STAGE == 37:
                            consume(rec[:])
                            continue
                        xn = wpool.tile([128, HID], dt.float32, name="xn", tag="xn")
                        nc.vector.tensor_tensor(
                            xn[:].rearrange("p (h c) -> p h c", c=C),
                            psw[:, 0:HID].rearrange("p (h c) -> p h c", c=C),
                            rec[:].unsqueeze(2).broadcast_to([128, 4, C]),
                            op=OP.mult,
                        )
                        nc.vector.tensor_tensor(xn[:], xn[:], gbr_sb[:, l, :], op=OP.add)
                        nc.vector.tensor_scalar_max(xn[:], xn[:], 0.0)
                        if STAGE == 38:
                            consume(xn[:])
                            continue
                        mus = wpool.tile([128, 1], dt.float32, name="mus", tag="mus")
                        nc.vector.reduce_sum(mus[:], xn[:], axis=AX.X)
                        mu = wpool.tile([128, 1], dt.float32, name="mu", tag="mu")
                        nc.scalar.mul(mu[:], mus[:], 1.0 / HID)
                        nc.vector.tensor_scalar_sub(xn[:], xn[:], mu[:])
                        if STAGE == 39:
                            consume(xn[:])
                            continue
                        sq = wpool.tile([128, HID], dt.float32, name="sq", tag="sq")
                        nc.any.tensor_tensor(sq[:], xn[:], xn[:], op=OP.mult)
                        nc.vector.reduce_sum(var_sb[:, w : w + 1], sq[:], axis=AX.X)
                        nc.any.tensor_tensor(
                            h_sb[:, w, :], xn[:], lgr_sb[:, l, :], op=OP.mult
                        )

                    # --- batched rsqrt + fixup + bias ---
                    if STAGE < 5 or _sub or STAGE in (52, 60, 61, 62, 63, 64, 65):
                        continue
                    sdv = b1pool.tile([128, NTILES], dt.float32, name="sdv", tag="sdv")
                    nc.scalar.activation(
                        sdv[:], var_sb[:], ACT.Sqrt, bias=eps1[:], scale=1.0 / HID
                    )
                    nc.vector.reciprocal(rstd_sb[:], sdv[:])
                    nc.vector.tensor_tensor(
                        h_sb[:], h_sb[:],
                        rstd_sb[:].unsqueeze(2).broadcast_to([128, NTILES, HID]),
                        op=OP.mult,
                    )
                    nc.vector.tensor_tensor(
                        h_sb[:], h_sb[:],
                        lbr_sb[:, l, :].unsqueeze(1).broadcast_to([128, NTILES, HID]),
                        op=OP.add,
                    )

                    # --- transposes for next layer ---
                    if l < L - 1 and not (60 <= STAGE <= 66):
                        for w in range(NTILES):
                            for k in range(2):
                                pst = tps.tile([128, 128], dt.bfloat16, name="pst", tag="pst")
                                nc.tensor.transpose(
                                    pst[:], h_sb[:, w, k * 128 : (k + 1) * 128], identb_sb[:]
                                )
                                nc.vector.tensor_copy(
                                    hT[:, k * PADSHARD + w * 128 : k * PADSHARD + (w + 1) * 128],
                                    pst[:],
                                )

            # ---- pooling + readout ----
            if STAGE < 98 and not STAGE == 7:
                with tc.tile_pool(name="dbg", bufs=1) as dbgp:
                    dbg = dbgp.tile([B, HID], dt.float32, name="dbg")
                    if (30 <= STAGE < 40) or STAGE == 52 or 60 <= STAGE <= 66:
                        nc.vector.tensor_copy(dbg[:], h_sb[0:B, 0, :])
                    else:
                        nc.vector.tensor_copy(dbg[:], hT[0:B, 0:HID])
                    nc.sync.dma_start(t_out.ap(), dbg[:])
            else:
              with (
                tc.tile_pool(name="ro", bufs=1) as ropool,
                tc.tile_pool(name="rops", bufs=1, space="PSUM") as rops,
              ):
                psp = rops.tile([B, HID], dt.float32, name="psp")
                for w in range(NTILES):
                    nc.tensor.matmul(
                        psp[:], ohb_sb[:, w, :], h_sb[:, w, :],
                        start=(w == 0), stop=(w == NTILES - 1),
                    )
                pp = ropool.tile([B, HID], dt.float32, name="pp")
                nc.vector.tensor_copy(pp[:], psp[:])
                nc.sync.dma_start(ar_in[:], pp[:])
                if STAGE == 98:
                    nc.sync.dma_start(ar_out[:, :], ar_in[:, :])
                else:
                    nc.gpsimd.collective_compute(
                        "AllReduce", OP.add, replica_groups=RG,
                        ins=[ar_in.opt()], outs=[ar_out.opt()],
                    )
                pooled = ropool.tile([B, HID], dt.float32, name="pooled")
                nc.sync.dma_start(pooled[:], ar_out[:])
                nc.vector.tensor_scalar_mul(pooled[:], pooled[:], invc_sb[:])

                pldT = ropool.tile([128, 2, B], dt.float32, name="pldT")
                for k in range(2):
                    pstf = rops.tile([128, B], dt.float32, name="pstf", tag="pstf")
                    nc.tensor.transpose(
                        pstf[:], pooled[:, k * 128 : (k + 1) * 128], identf_sb[0:B, 0:B]
                    )
                    nc.vector.tensor_copy(pldT[:, k, :], pstf[:])
                z1ps = rops.tile([B, HID], dt.float32, name="z1ps")
                for k in range(2):
                    nc.tensor.matmul(
                        z1ps[:], pldT[:, k, :], row1_sb[:, k, :],
                        start=(k == 0), stop=(k == 1),
                    )
                z1b = ropool.tile([B, HID], dt.float32, name="z1b")
                nc.vector.tensor_tensor(z1b[:], z1ps[:], b1r_sb[:], op=OP.add)
                z1g = ropool.tile([B, HID], dt.float32, name="z1g")
                gelu_fn = ACT.Identity if SIM_MODE else ACT.Gelu
                nc.scalar.activation(z1g[:], z1b[:], gelu_fn, bias=zero1[0:B, :])
                z1gT = ropool.tile([128, 2, B], dt.float32, name="z1gT")
                for k in range(2):
                    pstf2 = rops.tile([128, B], dt.float32, name="pstf2", tag="pstf")
                    nc.tensor.transpose(
                        pstf2[:], z1g[:, k * 128 : (k + 1) * 128], identf_sb[0:B, 0:B]
                    )
                    nc.vector.tensor_copy(z1gT[:, k, :], pstf2[:])
                z2ps = rops.tile([B, HID], dt.float32, name="z2ps")
                for k in range(2):
                    nc.tensor.matmul(
                        z2ps[:], z1gT[:, k, :], row2_sb[:, k, :],
                        start=(k == 0), stop=(k == 1),
                    )
                ob = ropool.tile([B, HID], dt.float32, name="ob")
                nc.vector.tensor_tensor(ob[:], z2ps[:], b2r_sb[:], op=OP.add)
                nc.sync.dma_start(t_out.ap(), ob[:])

    nc.compile()
    return nc


last_exec_ns = None
last_results = None


def _pjrt_timed(nc, in_maps, iters=30, chain=1):
    """Execute the compiled Bass graph via PJRT with device-resident inputs,
    returning (out_core0, per-iter wall times). chain>1 runs the NEFF that
    many times back-to-back inside one dispatch (output buffers fed forward)."""
    import time

    import jax
    from jax.experimental.shard_map import shard_map
    from jax.sharding import Mesh, NamedSharding, PartitionSpec

    from concourse import bass2jax, mybir

    bass2jax.install_neuronx_cc_hook()

    partition_name = nc.partition_id_tensor.name if nc.partition_id_tensor else None
    in_names, out_names, out_avals, zero_outs = [], [], [], []
    for alloc in nc.m.functions[0].allocations:
        if not isinstance(alloc, mybir.MemoryLocationSet):
            continue
        name = alloc.memorylocations[0].name
        if alloc.kind == "ExternalInput":
            if name != partition_name:
                in_names.append(name)
        elif alloc.kind == "ExternalOutput":
            out_names.append(name)
            shape = tuple(alloc.tensor_shape)
            dtype = mybir.dt.np(alloc.dtype)
            out_avals.append(jax.core.ShapedArray(shape, dtype))
            zero_outs.append(np.zeros(shape, dtype))
    n_params = len(in_names)
    all_names = list(in_names) + list(out_names)
    if partition_name is not None:
        all_names.append(partition_name)
    all_names = tuple(all_names)

    def _call(ins, zouts):
        pid = [bass2jax.partition_id_tensor()] if partition_name is not None else []
        return bass2jax._bass_exec_p.bind(
            *ins,
            *zouts,
            *pid,
            out_avals=tuple(out_avals),
            in_names=all_names,
            out_names=tuple(out_names),
            lowering_input_output_aliases=(),
            sim_require_finite=True,
            sim_require_nnan=True,
            nc=nc,
        )

    def _body(*args):
        ins = list(args[:n_params])
        zouts = list(args[n_params:])
        if chain == 1:
            return tuple(_call(ins, zouts))

        def step(carry, _):
            return list(_call(ins, carry)), 0

        carry, _ = jax.lax.scan(step, zouts, xs=None, length=chain)
        return tuple(carry)

    ncores = len(in_maps)
    devices = jax.devices()[:ncores]
    mesh = Mesh(np.asarray(devices), ("core",))
    in_specs = (PartitionSpec("core"),) * (n_params + len(out_names))
    out_specs = (PartitionSpec("core"),) * len(out_names)
    fn = jax.jit(
        shard_map(_body, mesh=mesh, in_specs=in_specs, out_specs=out_specs,
                  check_rep=False),
        keep_unused=True,
    )
    sh = NamedSharding(mesh, PartitionSpec("core"))
    dev_in = [
        jax.device_put(
            np.concatenate([np.asarray(in_maps[c][k]) for c in range(ncores)], axis=0), sh
        )
        for k in in_names
    ] + [
        jax.device_put(np.zeros((ncores * z.shape[0], *z.shape[1:]), z.dtype), sh)
        for z in zero_outs
    ]
    r = fn(*dev_in)
    jax.block_until_ready(r)
    times = []
    for _ in range(iters):
        t0 = time.perf_counter()
        r = fn(*dev_in)
        jax.block_until_ready(r)
        times.append(time.perf_counter() - t0)
    out = np.asarray(r[0]).reshape(ncores, *out_avals[0].shape)[0]
    return out, times


def bench(inputs, iters=30, chain=1):
    """Returns (out, times_list). Uses cached compiled graph."""
    in_maps, struct = _prep(inputs)
    key = (tuple(struct["T_lo"]), tuple(struct["T_hi"]))
    if key not in _cache:
        _cache[key] = _build(struct)
    return _pjrt_timed(_cache[key], in_maps, iters=iters, chain=chain)


def kernel(**inputs):
    global last_exec_ns, last_results
    from concourse import bass_utils

    in_maps, struct = _prep(inputs)
    key = (tuple(struct["T_lo"]), tuple(struct["T_hi"]))
    if key not in _cache:
        _cache[key] = _build(struct)
    nc = _cache[key]

    res = bass_utils.run_bass_kernel_spmd(
        nc, in_maps, core_ids=list(range(NCORES)), trace=False
    )
    last_exec_ns = res.exec_time_ns
    last_results = res
    return np.asarray(res.results[0]["out"], np.float32)


# revision 52
# speedup vs baseline: 21.4020x; 1.0780x over previous
"""Trainium2 Bass kernel for AIRS-GNN (4-layer GAT + readout) on 8 NeuronCores.

Self-contained: hardcodes all shapes/sharding. Host side does only integer
index manipulation (edge partitioning, padding, one-hot layout) plus constant
table construction; all floating-point math runs on device.

Sharding: nodes are sharded contiguously across the 8 cores (6250/core,
padded to 6272 = 49*128 rows).  Edges are owned by the core of their dst
node (pull-based aggregation).  Per layer the bf16 node-feature table
(h @ W) is AllGathered, then each core gathers the rows for its edges with
dma_gather and scatter-adds messages into per-window PSUM via one-hot
matmuls.  Softmax normalization is applied after aggregation (denominators
ride along as 4 extra matmul columns); the segment-max shift of the
reference cancels algebraically and is skipped.
"""

import os
import numpy as np
import ml_dtypes

# ---------------- problem constants (from spec) ----------------
N, E, F, HID, HEADS, L, B, R = 50000, 400000, 64, 256, 4, 4, 16, 8
C = HID // HEADS  # 64
NCORES = 8
SHARD = N // NCORES            # 6250
NTILES = (SHARD + 127) // 128  # 49
PADSHARD = NTILES * 128        # 6272
HALF = PADSHARD * (NCORES // 2)  # 25088 rows per half-table
LN_EPS = 1e-5
NEG_SLOPE = 0.2

BF16 = ml_dtypes.bfloat16

SIM_MODE = False  # debug: replace sim-unsupported Gelu with Identity
STAGE = 99  # debug: truncate graph after this stage (99 = full)

_cache = {}


def _posenc(n, d):
    pos = np.arange(n, dtype=np.float32)[:, None]
    i = np.arange(d, dtype=np.float32)[None, :]
    rates = (1.0 / 10000.0 ** (2.0 * np.floor(i / 2.0) / d)).astype(np.float32)
    ang = pos * rates
    return np.where(np.arange(d)[None, :] % 2 == 0, np.sin(ang), np.cos(ang)).astype(
        np.float32
    )


def _wrap16(a):
    """dma_gather index layout: idxs[p, s] = a[s*16 + p], replicated to 128 parts."""
    assert a.size % 16 == 0
    w = a.reshape(-1, 16).T.astype(np.int16)  # [16, S/16]
    return np.tile(w, (8, 1))  # [128, S/16]


def _prep(inputs):
    """Host-side integer prep. Returns (in_maps, struct)."""
    ei = np.asarray(inputs["edge_index"])
    src = np.concatenate([np.asarray(ei[0]), np.arange(N)]).astype(np.int64)
    dst = np.concatenate([np.asarray(ei[1]), np.arange(N)]).astype(np.int64)

    core = dst // SHARD
    dloc = dst - core * SHARD
    win = dloc // 128
    drel = dloc - win * 128
    grow = (src // SHARD) * PADSHARD + (src % SHARD)  # padded global table row
    half = (grow >= HALF).astype(np.int64)

    # tiles needed per (window, half): max over cores
    cnt = np.zeros((NCORES, NTILES, 2), np.int64)
    np.add.at(cnt, (core, win, half), 1)
    T = np.maximum((cnt.max(axis=0) + 127) // 128, 1)  # [NTILES, 2]
    T_lo, T_hi = T[:, 0], T[:, 1]
    Tw = T_lo + T_hi
    wslot0 = np.concatenate([[0], np.cumsum(Tw * 128)])
    TOTSLOT = int(wslot0[-1])

    gidx_maps, sdidx_maps, ids_maps = [], [], []
    for c in range(NCORES):
        sel = core == c
        wc, dc, gc, hc = win[sel], drel[sel], grow[sel], half[sel]
        key = wc * 2 + hc
        order = np.argsort(key, kind="stable")
        wc, dc, gc, hc = wc[order], dc[order], gc[order], hc[order]
        ks = key[order]
        grp_start = np.searchsorted(ks, np.arange(NTILES * 2), side="left")
        pos = np.arange(ks.size) - grp_start[ks]
        slot = wslot0[wc] + np.where(hc == 1, T_lo[wc] * 128, 0) + pos
        assert slot.max() < TOTSLOT

        g_val = np.zeros(TOTSLOT, np.int64)          # pad -> row 0 (valid)
        g_val[slot] = gc - hc * HALF
        sd_val = np.zeros(TOTSLOT, np.int64)
        sd_val[slot] = wc * 128 + dc
        id_val = np.full(TOTSLOT, -1.0, np.float32)  # pad -> -1 (no one-hot match)
        id_val[slot] = dc

        gcols, sdcols, idcols = [], [], []
        for w in range(NTILES):
            s0 = wslot0[w]
            nlo, nhi = int(T_lo[w]) * 128, int(T_hi[w]) * 128
            gcols.append(_wrap16(g_val[s0 : s0 + nlo]))
            gcols.append(_wrap16(g_val[s0 + nlo : s0 + nlo + nhi]))
            sdcols.append(_wrap16(sd_val[s0 : s0 + nlo + nhi]))
            idcols.append(id_val[s0 : s0 + nlo + nhi].reshape(-1, 128).T.astype(BF16))
        gidx_maps.append(np.concatenate(gcols, axis=1))
        sdidx_maps.append(np.concatenate(sdcols, axis=1))
        ids_maps.append(np.concatenate(idcols, axis=1))

    # ---------------- dense/static per-core tensors ----------------
    x = np.asarray(inputs["x"], np.float32)
    region_ids = np.asarray(inputs["region_ids"]).astype(np.int64)
    batch = np.asarray(inputs["batch"]).astype(np.int64)
    pe = _posenc(N, F)

    counts = np.bincount(batch, minlength=B).astype(np.float32)
    inv_cnt = (1.0 / np.maximum(counts, 1.0)).astype(np.float32)[:, None]

    w_in = np.asarray(inputs["in_proj_w"], np.float32)  # [3F, HID]
    gat_w = np.asarray(inputs["gat_w"], np.float32)     # [L, HID, HID]
    # [128, L*2, HID+8]: chunk (l,k) = [W rows | A_src blockdiag | A_dst blockdiag]
    # so one matmul emits [h2 | s_src | s_dst].
    a_s = np.asarray(inputs["att_src"], np.float32)  # [L, HEADS, C]
    a_d = np.asarray(inputs["att_dst"], np.float32)
    gatw_h = np.zeros((128, L * 2, HID + 8), np.float32)
    for l in range(L):
        # X[f0, h] = sum_c W[f0, h*C+c] * a[h, c]  (scores act on h2 = h @ W)
        W = gat_w[l]
        xs = np.einsum("fhc,hc->fh", W.reshape(HID, HEADS, C), a_s[l])
        xd = np.einsum("fhc,hc->fh", W.reshape(HID, HEADS, C), a_d[l])
        for k in range(2):
            rows = slice(k * 128, (k + 1) * 128)
            gatw_h[:, l * 2 + k, 0:HID] = W[rows, :]
            gatw_h[:, l * 2 + k, HID : HID + 4] = xs[rows, :]
            gatw_h[:, l * 2 + k, HID + 4 : HID + 8] = xd[rows, :]
    gatw_h = np.ascontiguousarray(gatw_h).astype(BF16)

    def rep128(a, d):  # [L, HID] -> [128, L, HID] replicated, dtype d
        return np.ascontiguousarray(
            np.broadcast_to(np.asarray(a, np.float32)[None, :, :], (128, L, HID))
        ).astype(d)

    gbr_h = rep128(inputs["gat_b"], np.float32)
    lgr_h = rep128(inputs["ln_g"], np.float32)
    lbr_h = rep128(inputs["ln_b"], BF16)
    ipb_h = np.ascontiguousarray(
        np.asarray(inputs["in_proj_b"], np.float32).reshape(2, 128).T
    )  # [128, 2]
    row1_h = np.ascontiguousarray(
        np.asarray(inputs["ro_w1"], np.float32).reshape(2, 128, HID).transpose(1, 0, 2)
    )  # [128, 2, HID]
    row2_h = np.ascontiguousarray(
        np.asarray(inputs["ro_w2"], np.float32).reshape(2, 128, HID).transpose(1, 0, 2)
    )

    in_maps = []
    for c in range(NCORES):
        lo, hi = c * SHARD, (c + 1) * SHARD
        xT = np.zeros((F, PADSHARD), np.float32)
        xT[:, :SHARD] = x[lo:hi].T
        peT = np.zeros((F, PADSHARD), np.float32)
        peT[:, :SHARD] = pe[lo:hi].T
        rT = np.zeros((R, PADSHARD), np.float32)
        rT[region_ids[lo:hi], np.arange(SHARD)] = 1.0
        ohb = np.zeros((128, NTILES, B), BF16)
        p_all = np.arange(SHARD)
        ohb[p_all % 128, p_all // 128, batch[lo:hi]] = 1.0

        m = {
            "xT": xT,
            "peT": peT,
            "rT": rT,
            "gidx": gidx_maps[c].astype(np.int16),
            "sdidx": sdidx_maps[c].astype(np.int16),
            "ids": ids_maps[c].astype(BF16),
            "ohb": ohb,
            "w_x": w_in[:F].copy(),
            "w_r2": w_in[F : 2 * F].copy(),
            "w_p": w_in[2 * F :].copy(),
            "embT": np.asarray(inputs["region_emb_w"], np.float32).T.copy(),
            "ipb": ipb_h,
            "gatw": gatw_h,
            "gbr": gbr_h,
            "lgr": lgr_h,
            "lbr": lbr_h,
            "row1": row1_h,
            "row2": row2_h,
            "b1r": np.ascontiguousarray(
                np.broadcast_to(np.asarray(inputs["ro_b1"], np.float32)[None, :], (B, HID))
            ),
            "b2r": np.ascontiguousarray(
                np.broadcast_to(np.asarray(inputs["ro_b2"], np.float32)[None, :], (B, HID))
            ),
            "invc": inv_cnt,
            "iota": np.ascontiguousarray(
                np.broadcast_to(np.arange(128, dtype=np.float32)[None, :], (128, 128))
            ).astype(BF16),
            "identb": np.eye(128, dtype=np.float32).astype(BF16),
            "identf": np.eye(128, dtype=np.float32),
        }
        in_maps.append(m)

    struct = {
        "T_lo": [int(t) for t in T_lo],
        "T_hi": [int(t) for t in T_hi],
        "GCOLS": int(gidx_maps[0].shape[1]),
        "SDCOLS": int(sdidx_maps[0].shape[1]),
        "IDCOLS": int(ids_maps[0].shape[1]),
    }
    return in_maps, struct


def _build(struct):
    """Build the Bass graph (identical for all cores)."""
    import concourse.bass as bass  # noqa: F401
    import concourse.tile as tile
    from concourse import bacc, mybir

    dt = mybir.dt
    AX = mybir.AxisListType
    OP = mybir.AluOpType
    ACT = mybir.ActivationFunctionType

    T_lo, T_hi = struct["T_lo"], struct["T_hi"]
    Tw = [a + b for a, b in zip(T_lo, T_hi)]
    TMAX = max(Tw)

    nc = bacc.Bacc(
        "TRN2", target_bir_lowering=False, debug=False,
        num_devices=(1 if STAGE == 98 else NCORES),
    )
    RG = [list(range(NCORES))]

    def din(name, shape, d=dt.float32):
        return nc.dram_tensor(name, shape, d, kind="ExternalInput")

    t_xT = din("xT", [F, PADSHARD])
    t_peT = din("peT", [F, PADSHARD])
    t_rT = din("rT", [R, PADSHARD])
    t_gidx = din("gidx", [128, struct["GCOLS"]], dt.int16)
    t_sdidx = din("sdidx", [128, struct["SDCOLS"]], dt.int16)
    t_ids = din("ids", [128, struct["IDCOLS"]], dt.bfloat16)
    t_ohb = din("ohb", [128, NTILES, B], dt.bfloat16)
    t_wx = din("w_x", [F, HID])
    t_wr2 = din("w_r2", [F, HID])
    t_wp = din("w_p", [F, HID])
    t_embT = din("embT", [F, R])
    t_ipb = din("ipb", [128, 2])
    t_gatw = din("gatw", [128, L * 2, HID + 8], dt.bfloat16)
    t_gbr = din("gbr", [128, L, HID])
    t_lgr = din("lgr", [128, L, HID])
    t_lbr = din("lbr", [128, L, HID], dt.bfloat16)
    t_row1 = din("row1", [128, 2, HID])
    t_row2 = din("row2", [128, 2, HID])
    t_b1r = din("b1r", [B, HID])
    t_b2r = din("b2r", [B, HID])
    t_invc = din("invc", [B, 1])
    t_iota = din("iota", [128, 128], dt.bfloat16)
    t_identb = din("identb", [128, 128], dt.bfloat16)
    t_identf = din("identf", [128, 128])

    t_out = nc.dram_tensor("out", [B, HID], dt.float32, kind="ExternalOutput")

    # static per-window offsets (in idx columns / id columns)
    gcall = [0]
    for w in range(NTILES):
        gcall.append(gcall[-1] + T_lo[w] * 8)
        gcall.append(gcall[-1] + T_hi[w] * 8)
    sdoff = [0]
    idoff = [0]
    for w in range(NTILES):
        sdoff.append(sdoff[-1] + Tw[w] * 8)
        idoff.append(idoff[-1] + Tw[w])

    with tile.TileContext(nc) as tc:
        with (
            tc.tile_pool(name="const", bufs=1) as cpool,
            tc.tile_pool(name="dram", bufs=1, space="DRAM") as dpool,
            tc.tile_pool(name="persist", bufs=1) as ppool,
        ):
            def load(t, shape, d=dt.float32):
                tl = cpool.tile(shape, d, name=t.name + "_sb")
                nc.sync.dma_start(tl[:], t.ap())
                return tl

            gidx_sb = load(t_gidx, [128, struct["GCOLS"]], dt.int16)
            sdidx_sb = load(t_sdidx, [128, struct["SDCOLS"]], dt.int16)
            ids_sb = load(t_ids, [128, struct["IDCOLS"]], dt.bfloat16)
            ohb_sb = load(t_ohb, [128, NTILES, B], dt.bfloat16)
            ipb_sb = load(t_ipb, [128, 2])
            gatw_sb = load(t_gatw, [128, L * 2, HID + 8], dt.bfloat16)
            gbr_sb = load(t_gbr, [128, L, HID])
            lgr_sb = load(t_lgr, [128, L, HID])
            lbr_sb = load(t_lbr, [128, L, HID], dt.bfloat16)
            row1_sb = load(t_row1, [128, 2, HID])
            row2_sb = load(t_row2, [128, 2, HID])
            b1r_sb = load(t_b1r, [B, HID])
            b2r_sb = load(t_b2r, [B, HID])
            invc_sb = load(t_invc, [B, 1])
            iota_sb = load(t_iota, [128, 128], dt.bfloat16)
            identb_sb = load(t_identb, [128, 128], dt.bfloat16)
            identf_sb = load(t_identf, [128, 128])

            zero1 = cpool.tile([128, 1], dt.float32, name="zero1")
            nc.vector.memset(zero1[:], 0.0)
            eps1 = cpool.tile([128, 1], dt.float32, name="eps1")
            nc.vector.memset(eps1[:], LN_EPS)

            hT = ppool.tile([128, 2 * PADSHARD], dt.bfloat16, name="hT")
            h_sb = ppool.tile([128, NTILES, HID], dt.bfloat16, name="h_sb")
            var_sb = ppool.tile([128, NTILES], dt.float32, name="var_sb")
            rstd_sb = ppool.tile([128, NTILES], dt.float32, name="rstd_sb")

            GW = HID + 128  # 384-col rows: [h2 | s_src | pad], 768B
            in_cc = [
                dpool.tile([PADSHARD, GW], dt.bfloat16, name=f"incc{l}")
                for l in range(L)
            ]
            out_cc = [
                dpool.tile(
                    [NCORES * PADSHARD, GW], dt.bfloat16, name=f"outcc{l}",
                    addr_space="Shared",
                )
                for l in range(L)
            ]
            s_dram = [
                dpool.tile([PADSHARD, 128], dt.bfloat16, name=f"sdram{l}")
                for l in range(L)
            ]
            ar_in = dpool.tile([B, HID], dt.float32, name="ar_in")
            ar_out = dpool.tile([B, HID], dt.float32, name="ar_out", addr_space="Shared")

            # ---- stage A: input projection -> hT (bf16) ----
            with (
                tc.tile_pool(name="aproj", bufs=1) as apool,
                tc.tile_pool(name="apsum", bufs=2, space="PSUM") as appool,
            ):
                xT_sb = apool.tile([F, PADSHARD], dt.float32, name="xT_sb")
                nc.sync.dma_start(xT_sb[:], t_xT.ap())
                peT_sb = apool.tile([F, PADSHARD], dt.float32, name="peT_sb")
                nc.sync.dma_start(peT_sb[:], t_peT.ap())
                rT_sb = apool.tile([R, PADSHARD], dt.float32, name="rT_sb")
                nc.sync.dma_start(rT_sb[:], t_rT.ap())
                wx_sb = apool.tile([F, HID], dt.float32, name="wx_sb")
                nc.sync.dma_start(wx_sb[:], t_wx.ap())
                wr2_sb = apool.tile([F, HID], dt.float32, name="wr2_sb")
                nc.sync.dma_start(wr2_sb[:], t_wr2.ap())
                wp_sb = apool.tile([F, HID], dt.float32, name="wp_sb")
                nc.sync.dma_start(wp_sb[:], t_wp.ap())
                embT_sb = apool.tile([F, R], dt.float32, name="embT_sb")
                nc.sync.dma_start(embT_sb[:], t_embT.ap())

                ew_ps = appool.tile([R, HID], dt.float32, name="ew_ps")
                nc.tensor.matmul(ew_ps[:], embT_sb[:], wr2_sb[:])
                ew_sb = apool.tile([R, HID], dt.float32, name="ew_sb")
                nc.vector.tensor_copy(ew_sb[:], ew_ps[:])

                NBLK = 512
                nblocks = (PADSHARD + NBLK - 1) // NBLK
                for k in range(2):
                    fs = slice(k * 128, (k + 1) * 128)
                    for nb in range(nblocks):
                        c0 = nb * NBLK
                        cw = min(NBLK, PADSHARD - c0)
                        ps = appool.tile([128, NBLK], dt.float32, name="aps", tag="aps")
                        nc.tensor.matmul(
                            ps[:, :cw], wx_sb[:, fs], xT_sb[:, c0 : c0 + cw],
                            start=True, stop=False,
                        )
                        nc.tensor.matmul(
                            ps[:, :cw], ew_sb[:, fs], rT_sb[:, c0 : c0 + cw],
                            start=False, stop=False,
                        )
                        nc.tensor.matmul(
                            ps[:, :cw], wp_sb[:, fs], peT_sb[:, c0 : c0 + cw],
                            start=False, stop=True,
                        )
                        nc.vector.tensor_scalar_add(
                            hT[:, k * PADSHARD + c0 : k * PADSHARD + c0 + cw],
                            ps[:, :cw],
                            ipb_sb[:, k : k + 1],
                        )

            # ---- GAT layers ----
            with (
                tc.tile_pool(name="b1", bufs=3) as b1pool,
                tc.tile_pool(name="b1ps", bufs=2, space="PSUM") as b1ps,
                tc.tile_pool(name="win", bufs=4) as wpool,
                tc.tile_pool(name="win2", bufs=2) as wpool2,
                tc.tile_pool(name="wps", bufs=3, space="PSUM") as wps,
                tc.tile_pool(name="tps", bufs=2, space="PSUM") as tps,
            ):
                _sub = 20 <= STAGE < 40
                if STAGE in (50, 51, 52, 98) or 60 <= STAGE <= 66:
                    nlayers = L
                elif STAGE < 1:
                    nlayers = 0
                elif STAGE < 6 or _sub:
                    nlayers = 1
                else:
                    nlayers = L
                for l in range(nlayers):
                    # --- B1: h2 = h @ W_l; s_dst; feature table ---
                    for w in range(NTILES):
                        ps2 = b1ps.tile([128, HID + 8], dt.float32, name="h2ps", tag="h2ps")
                        for k in range(2):
                            nc.tensor.matmul(
                                ps2[:],
                                hT[:, k * PADSHARD + w * 128 : k * PADSHARD + (w + 1) * 128],
                                gatw_sb[:, l * 2 + k, :],
                                start=(k == 0),
                                stop=(k == 1),
                            )
                        g_t = b1pool.tile([128, GW], dt.bfloat16, name="g_t", tag="g_t")
                        if SIM_MODE:
                            nc.vector.memset(g_t[:], 0.0)
                        nc.vector.tensor_copy(g_t[:, 0 : HID + 4], ps2[:, 0 : HID + 4])
                        nc.sync.dma_start(in_cc[l][w * 128 : (w + 1) * 128, :], g_t[:])
                        sdrow = b1pool.tile([128, 128], dt.bfloat16, name="sdrow", tag="sdrow")
                        if SIM_MODE:
                            nc.vector.memset(sdrow[:], 0.0)
                        nc.vector.tensor_copy(sdrow[:, 0:4], ps2[:, HID + 4 : HID + 8])
                        nc.sync.dma_start(s_dram[l][w * 128 : (w + 1) * 128, :], sdrow[:])

                    # --- B2: AllGather feature table ---
                    if STAGE < 2 or STAGE == 51:
                        continue
                    if STAGE == 98:
                        nc.sync.dma_start(out_cc[l][0:PADSHARD, :], in_cc[l][:, :])
                    else:
                        nc.gpsimd.collective_compute(
                            "AllGather", OP.bypass, replica_groups=RG,
                            ins=[in_cc[l].opt()], outs=[out_cc[l].opt()],
                        )
                    if STAGE < 3 or STAGE == 50:
                        continue

                    # --- B3: windows ---
                    nwin = NTILES if ((STAGE >= 5 and not _sub) or STAGE in (52, 98) or 60 <= STAGE <= 66) else 1
                    for w in range(nwin):
                        tl, th = T_lo[w], T_hi[w]
                        tw = tl + th
                        g = wpool.tile([128, TMAX, GW], dt.bfloat16, name="g", tag="g")
                        sd = wpool.tile([128, TMAX, 128], dt.bfloat16, name="sd", tag="sd")
                        en_glo = STAGE not in (21, 22, 23)
                        en_ghi = STAGE not in (20, 22, 23)
                        en_sd = STAGE not in (20, 21, 22)
                        if STAGE in (20, 21, 22, 23):
                            nc.vector.memset(g[:], 0.0)
                            nc.vector.memset(sd[:], 0.0)
                        if en_glo:
                            nc.gpsimd.dma_gather(
                                g[:, 0:tl, :],
                                out_cc[l][0:HALF, :],
                                gidx_sb[:, gcall[2 * w] : gcall[2 * w] + tl * 8],
                                num_idxs=tl * 128,
                                num_idxs_reg=tl * 128,
                                elem_size=GW,
                                single_packet=False,
                            )
                        if en_ghi:
                            nc.gpsimd.dma_gather(
                                g[:, tl:tw, :],
                                out_cc[l][HALF : 2 * HALF, :],
                                gidx_sb[:, gcall[2 * w + 1] : gcall[2 * w + 1] + th * 8],
                                num_idxs=th * 128,
                                num_idxs_reg=th * 128,
                                elem_size=GW,
                                single_packet=False,
                            )
                        if en_sd:
                            nc.gpsimd.dma_gather(
                                sd[:, 0:tw, :],
                                s_dram[l][:, :],
                                sdidx_sb[:, sdoff[w] : sdoff[w] + tw * 8],
                                num_idxs=tw * 128,
                                num_idxs_reg=tw * 128,
                                elem_size=128,
                                single_packet=False,
                            )
                        def consume(ap):
                            nc.vector.tensor_copy(h_sb[:, w, 0 : ap.shape[-1]], ap)

                        if STAGE in (20, 21, 22, 23, 52):
                            consume(g[:, 0, 0:HID])
                            consume(sd[:, 0, 0:128])
                            continue
                        if STAGE == 30:
                            consume(g[:, 0, 0:HID])
                            continue
                        if STAGE == 31:
                            consume(g[:, 0, 0:HID])
                            consume(sd[:, 0, 0:128])
                            continue
                        if STAGE < 4:
                            continue
                        if STAGE in (32, 60):
                            consume(g[:, 0, HID : HID + 4])
                            continue
                        ef = wpool.tile([128, TMAX, 4], dt.float32, name="ef", tag="ef")
                        nc.vector.tensor_tensor(
                            ef[:, 0:tw, :], g[:, 0:tw, HID : HID + 4],
                            sd[:, 0:tw, 0:4], op=OP.add
                        )
                        e2 = wpool.tile([128, TMAX, 4], dt.float32, name="e2", tag="e2")
                        nc.vector.tensor_scalar_mul(e2[:, 0:tw, :], ef[:, 0:tw, :], NEG_SLOPE)
                        nc.vector.tensor_tensor(
                            e2[:, 0:tw, :], ef[:, 0:tw, :], e2[:, 0:tw, :], op=OP.max
                        )
                        ex = wpool.tile([128, TMAX, 4], dt.float32, name="ex", tag="ex")
                        nc.scalar.activation(
                            ex[:, 0:tw, :], e2[:, 0:tw, :], ACT.Exp, bias=zero1[:]
                        )
                        if STAGE in (33, 61):
                            consume(ex[:, 0, 0:4])
                            continue
                        oh = wpool.tile([128, TMAX, 128], dt.bfloat16, name="oh", tag="oh")
                        nc.any.tensor_tensor(
                            oh[:, 0:tw, :],
                            ids_sb[:, idoff[w] : idoff[w] + tw]
                            .unsqueeze(2).broadcast_to([128, tw, 128]),
                            iota_sb[:].unsqueeze(1).broadcast_to([128, tw, 128]),
                            op=OP.is_equal,
                        )
                        if STAGE in (34, 62):
                            consume(oh[:, 0, 0:128])
                            continue
                        rhsw = wpool2.tile(
                            [128, TMAX, HID + 4], dt.bfloat16, name="rhsw", tag="rhsw"
                        )
                        nc.any.tensor_tensor(
                            rhsw[:, 0:tw, 0:HID].rearrange("p t (h c) -> p t h c", c=C),
                            g[:, 0:tw, 0:HID].rearrange("p t (h c) -> p t h c", c=C),
                            ex[:, 0:tw, :].unsqueeze(3).broadcast_to([128, tw, HEADS, C]),
                            op=OP.mult,
                        )
                        nc.any.tensor_copy(rhsw[:, 0:tw, HID : HID + 4], ex[:, 0:tw, :])
                        if STAGE in (35, 63):
                            consume(rhsw[:, 0, 0:HID])
                            continue
                        psw = wps.tile([128, HID + 4], dt.float32, name="psw", tag="psw")
                        for t in range(tw):
                            nc.tensor.matmul(
                                psw[:], oh[:, t, :], rhsw[:, t, :],
                                start=(t == 0), stop=(t == tw - 1),
                            )
                        if STAGE in (36, 64):
                            consume(psw[:, 0:HID])
                            continue
                        # epilogue
                        den = wpool.tile([128, 4], dt.float32, name="den", tag="den")
                        nc.vector.tensor_scalar_max(den[:], psw[:, HID : HID + 4], 1e-30)
                        rec = wpool.tile([128, 4], dt.float32, name="rec", tag="rec")
                        nc.vector.reciprocal(rec[:], den[:])
                        if STAGE == 37:
                            consume(rec[:])
                            continue
                        xn = wpool.tile([128, HID], dt.float32, name="xn", tag="xn")
                        nc.vector.tensor_tensor(
                            xn[:].rearrange("p (h c) -> p h c", c=C),
                            psw[:, 0:HID].rearrange("p (h c) -> p h c", c=C),
                            rec[:].unsqueeze(2).broadcast_to([128, 4, C]),
                            op=OP.mult,
                        )
                        nc.vector.tensor_tensor(xn[:], xn[:], gbr_sb[:, l, :], op=OP.add)
                        nc.vector.tensor_scalar_max(xn[:], xn[:], 0.0)
                        if STAGE == 38:
                            consume(xn[:])
                            continue
                        mus = wpool.tile([128, 1], dt.float32, name="mus", tag="mus")
                        nc.vector.reduce_sum(mus[:], xn[:], axis=AX.X)
                        mu = wpool.tile([128, 1], dt.float32, name="mu", tag="mu")
                        nc.scalar.mul(mu[:], mus[:], 1.0 / HID)
                        nc.vector.tensor_scalar_sub(xn[:], xn[:], mu[:])
                        if STAGE == 39:
                            consume(xn[:])
                            continue
                        sq = wpool.tile([128, HID], dt.float32, name="sq", tag="sq")
                        nc.any.tensor_tensor(sq[:], xn[:], xn[:], op=OP.mult)
                        nc.vector.reduce_sum(var_sb[:, w : w + 1], sq[:], axis=AX.X)
                        nc.any.tensor_tensor(
                            h_sb[:, w, :], xn[:], lgr_sb[:, l, :], op=OP.mult
                        )

                    # --- batched rsqrt + fixup + bias ---
                    if STAGE < 5 or _sub or STAGE in (52, 60, 61, 62, 63, 64, 65):
                        continue
                    sdv = b1pool.tile([128, NTILES], dt.float32, name="sdv", tag="sdv")
                    nc.scalar.activation(
                        sdv[:], var_sb[:], ACT.Sqrt, bias=eps1[:], scale=1.0 / HID
                    )
                    nc.vector.reciprocal(rstd_sb[:], sdv[:])
                    nc.vector.tensor_tensor(
                        h_sb[:], h_sb[:],
                        rstd_sb[:].unsqueeze(2).broadcast_to([128, NTILES, HID]),
                        op=OP.mult,
                    )
                    nc.vector.tensor_tensor(
                        h_sb[:], h_sb[:],
                        lbr_sb[:, l, :].unsqueeze(1).broadcast_to([128, NTILES, HID]),
                        op=OP.add,
                    )

                    # --- transposes for next layer ---
                    if l < L - 1 and not (60 <= STAGE <= 66):
                        for w in range(NTILES):
                            for k in range(2):
                                pst = tps.tile([128, 128], dt.bfloat16, name="pst", tag="pst")
                                nc.tensor.transpose(
                                    pst[:], h_sb[:, w, k * 128 : (k + 1) * 128], identb_sb[:]
                                )
                                nc.vector.tensor_copy(
                                    hT[:, k * PADSHARD + w * 128 : k * PADSHARD + (w + 1) * 128],
                                    pst[:],
                                )

            # ---- pooling + readout ----
            if STAGE < 98 and not STAGE == 7:
                with tc.tile_pool(name="dbg", bufs=1) as dbgp:
                    dbg = dbgp.tile([B, HID], dt.float32, name="dbg")
                    if (30 <= STAGE < 40) or STAGE == 52 or 60 <= STAGE <= 66:
                        nc.vector.tensor_copy(dbg[:], h_sb[0:B, 0, :])
                    else:
                        nc.vector.tensor_copy(dbg[:], hT[0:B, 0:HID])
                    nc.sync.dma_start(t_out.ap(), dbg[:])
            else:
              with (
                tc.tile_pool(name="ro", bufs=1) as ropool,
                tc.tile_pool(name="rops", bufs=1, space="PSUM") as rops,
              ):
                psp = rops.tile([B, HID], dt.float32, name="psp")
                for w in range(NTILES):
                    nc.tensor.matmul(
                        psp[:], ohb_sb[:, w, :], h_sb[:, w, :],
                        start=(w == 0), stop=(w == NTILES - 1),
                    )
                pp = ropool.tile([B, HID], dt.float32, name="pp")
                nc.vector.tensor_copy(pp[:], psp[:])
                nc.sync.dma_start(ar_in[:], pp[:])
                if STAGE == 98:
                    nc.sync.dma_start(ar_out[:, :], ar_in[:, :])
                else:
                    nc.gpsimd.collective_compute(
                        "AllReduce", OP.add, replica_groups=RG,
                        ins=[ar_in.opt()], outs=[ar_out.opt()],
                    )
                pooled = ropool.tile([B, HID], dt.float32, name="pooled")
                nc.sync.dma_start(pooled[:], ar_out[:])
                nc.vector.tensor_scalar_mul(pooled[:], pooled[:], invc_sb[:])

                pldT = ropool.tile([128, 2, B], dt.float32, name="pldT")
                for k in range(2):
                    pstf = rops.tile([128, B], dt.float32, name="pstf", tag="pstf")
                    nc.tensor.transpose(
                        pstf[:], pooled[:, k * 128 : (k + 1) * 128], identf_sb[0:B, 0:B]
                    )
                    nc.vector.tensor_copy(pldT[:, k, :], pstf[:])
                z1ps = rops.tile([B, HID], dt.float32, name="z1ps")
                for k in range(2):
                    nc.tensor.matmul(
                        z1ps[:], pldT[:, k, :], row1_sb[:, k, :],
                        start=(k == 0), stop=(k == 1),
                    )
                z1b = ropool.tile([B, HID], dt.float32, name="z1b")
                nc.vector.tensor_tensor(z1b[:], z1ps[:], b1r_sb[:], op=OP.add)
                z1g = ropool.tile([B, HID], dt.float32, name="z1g")
                gelu_fn = ACT.Identity if SIM_MODE else ACT.Gelu
                nc.scalar.activation(z1g[:], z1b[:], gelu_fn, bias=zero1[0:B, :])
                z1gT = ropool.tile([128, 2, B], dt.float32, name="z1gT")
                for k in range(2):
                    pstf2 = rops.tile([128, B], dt.float32, name="pstf2", tag="pstf")
                    nc.tensor.transpose(
                        pstf2[:], z1g[:, k * 128 : (k + 1) * 128], identf_sb[0:B, 0:B]
                    )
                    nc.vector.tensor_copy(z1gT[:, k, :], pstf2[:])
                z2ps = rops.tile([B, HID], dt.float32, name="z2ps")
                for k in range(2):
                    nc.tensor.matmul(
                        z2ps[:], z1gT[:, k, :], row2_sb[:, k, :],
                        start=(k == 0), stop=(k == 1),
                    )
                ob = ropool.tile([B, HID], dt.float32, name="ob")
                nc.vector.tensor_tensor(ob[:], z2ps[:], b2r_sb[:], op=OP.add)
                nc.sync.dma_start(t_out.ap(), ob[:])

    nc.compile()
    return nc


last_exec_ns = None
last_results = None


def _pjrt_timed(nc, in_maps, iters=30, chain=1):
    """Execute the compiled Bass graph via PJRT with device-resident inputs,
    returning (out_core0, per-iter wall times). chain>1 runs the NEFF that
    many times back-to-back inside one dispatch (output buffers fed forward)."""
    import time

    import jax
    from jax.experimental.shard_map import shard_map
    from jax.sharding import Mesh, NamedSharding, PartitionSpec

    from concourse import bass2jax, mybir

    bass2jax.install_neuronx_cc_hook()

    partition_name = nc.partition_id_tensor.name if nc.partition_id_tensor else None
    in_names, out_names, out_avals, zero_outs = [], [], [], []
    for alloc in nc.m.functions[0].allocations:
        if not isinstance(alloc, mybir.MemoryLocationSet):
            continue
        name = alloc.memorylocations[0].name
        if alloc.kind == "ExternalInput":
            if name != partition_name:
                in_names.append(name)
        elif alloc.kind == "ExternalOutput":
            out_names.append(name)
            shape = tuple(alloc.tensor_shape)
            dtype = mybir.dt.np(alloc.dtype)
            out_avals.append(jax.core.ShapedArray(shape, dtype))
            zero_outs.append(np.zeros(shape, dtype))
    n_params = len(in_names)
    all_names = list(in_names) + list(out_names)
    if partition_name is not None:
        all_names.append(partition_name)
    all_names = tuple(all_names)

    def _call(ins, zouts):
        pid = [bass2jax.partition_id_tensor()] if partition_name is not None else []
        return bass2jax._bass_exec_p.bind(
            *ins,
            *zouts,
            *pid,
            out_avals=tuple(out_avals),
            in_names=all_names,
            out_names=tuple(out_names),
            lowering_input_output_aliases=(),
            sim_require_finite=True,
            sim_require_nnan=True,
            nc=nc,
        )

    def _body(*args):
        ins = list(args[:n_params])
        zouts = list(args[n_params:])
        if chain == 1:
            return tuple(_call(ins, zouts))

        def step(carry, _):
            return list(_call(ins, carry)), 0

        carry, _ = jax.lax.scan(step, zouts, xs=None, length=chain)
        return tuple(carry)

    ncores = len(in_maps)
    devices = jax.devices()[:ncores]
    mesh = Mesh(np.asarray(devices), ("core",))
    in_specs = (PartitionSpec("core"),) * (n_params + len(out_names))
    out_specs = (PartitionSpec("core"),) * len(out_names)
    fn = jax.jit(
        shard_map(_body, mesh=mesh, in_specs=in_specs, out_specs=out_specs,
                  check_rep=False),
        keep_unused=True,
    )
    sh = NamedSharding(mesh, PartitionSpec("core"))
    dev_in = [
        jax.device_put(
            np.concatenate([np.asarray(in_maps[c][k]) for c in range(ncores)], axis=0), sh
        )
        for k in in_names
    ] + [
        jax.device_put(np.zeros((ncores * z.shape[0], *z.shape[1:]), z.dtype), sh)
        for z in zero_outs
    ]
    r = fn(*dev_in)
    jax.block_until_ready(r)
    times = []
    for _ in range(iters):
        t0 = time.perf_counter()
        r = fn(*dev_in)
        jax.block_until_ready(r)
        times.append(time.perf_counter() - t0)
    out = np.asarray(r[0]).reshape(ncores, *out_avals[0].shape)[0]
    return out, times


def bench(inputs, iters=30, chain=1):
    """Returns (out, times_list). Uses cached compiled graph."""
    in_maps, struct = _prep(inputs)
    key = (tuple(struct["T_lo"]), tuple(struct["T_hi"]))
    if key not in _cache:
        _cache[key] = _build(struct)
    return _pjrt_timed(_cache[key], in_maps, iters=iters, chain=chain)


def kernel(**inputs):
    global last_exec_ns, last_results
    from concourse import bass_utils

    in_maps, struct = _prep(inputs)
    key = (tuple(struct["T_lo"]), tuple(struct["T_hi"]))
    if key not in _cache:
        _cache[key] = _build(struct)
    nc = _cache[key]

    res = bass_utils.run_bass_kernel_spmd(
        nc, in_maps, core_ids=list(range(NCORES)), trace=False
    )
    last_exec_ns = res.exec_time_ns
    last_results = res
    return np.asarray(res.results[0]["out"], np.float32)
